# revision 1
# baseline (speedup 1.0000x reference)
"""Causal attention (B=4, T=4096, D=256) on 8 TRN2 NeuronCores.

Sharding: data-parallel over batch x query-halves. Core c handles batch
b = c//2 and query half h = c%2. The active builder (v3, VERSION=3)
groups queries 512 wide: group g of core h owns the interleaved global
128-row query tiles {8g + 2u + h : u in 0..3}, so both halves see the
same s-extent (8g+8 tiles) per group. That makes causal work exactly
balanced AND the program SPMD-uniform: every core runs the identical
instruction stream; only the input DATA (gathered query rows, per-core
0/1 mask blocks) differs.

On-chip layout (flash-attention style, nothing T^2-sized touches HBM):
  xT  [d, t]  (bf16)  -> QT/KT projections directly in transposed layout
                         (lhsT = W [din, dout], rhs = xT)
  S^T [s, q] = matmul(lhsT=KT_tile, rhs=QT_[512 q cols])  (contract d)
  P^T = exp(scale * S^T)  on ACT (no max-subtraction: logits are O(1),
        exp cannot overflow in fp32), then a multiplicative 0/1 bf16
        mask on the single diagonal 128-col block (DVE)
  O   [q, d] = sum_s matmul(lhsT=P^T 128-col slice, rhs=V_ext)
where V_ext has a ones column appended, so O[:, D] accumulates the
softmax denominator for free; final divide is a per-partition scalar.
S^T/exp are narrowed to live columns in the diagonal region and PV
matmuls that are dead on BOTH cores are skipped (both SPMD-uniform).
A short garbage-matmul warm-up bridges the input-DMA wait so the PE
HAM clock gate is at 8/8 when real work arrives, and the single sync
HWDGE queue streams inputs in consumption order (weights, first x
chunks, rest, masks last). V is stored bias-free (bv is added in the
finalize: O/den + bv == using V+bv, since the P rows sum to den), so
the V PSUM->SBUF cast-copies run on the otherwise-idle ACT engine.
Measured ~113-116 us on hardware (8 cores, run-to-run drift ~2 us),
rel err ~2.3e-3 vs the fp32 reference (bf16 matmul precision).
"""

import os
import sys

import numpy as np

for _p in ("/opt/trn_rl_repo", "/root/.axon_site/_ro/trn_rl_repo"):
    if os.path.isdir(_p) and _p not in sys.path:
        sys.path.insert(0, _p)

import ml_dtypes  # noqa: E402

import concourse.bass as bass  # noqa: E402
import concourse.bacc as bacc  # noqa: E402
import concourse.mybir as mybir  # noqa: E402
import concourse.tile as tile  # noqa: E402
from concourse.bass_utils import run_bass_kernel_spmd  # noqa: E402

BF16 = mybir.dt.bfloat16
F32 = mybir.dt.float32
NPBF16 = ml_dtypes.bfloat16

B = 4
T = 4096
D = 256
N_CORES = 8
TQ = T // 2  # query rows per core
NEG = -1.0e9


def build_nc(t: int = T, tq: int = TQ) -> bass.Bass:
    nq = tq // 128  # query tiles per core
    ns = t // 128  # total key tiles
    assert t == 2 * tq and ns == 2 * nq
    scale = 1.0 / float(np.sqrt(np.float32(D)))

    nc = bacc.Bacc()
    xT_d = nc.dram_tensor("xT", [2, 128, t], BF16, kind="ExternalInput")
    xqT_d = nc.dram_tensor("xqT", [2, 128, tq], BF16, kind="ExternalInput")
    wq_d = nc.dram_tensor("wq", [2, 128, D], BF16, kind="ExternalInput")
    wk_d = nc.dram_tensor("wk", [2, 128, D], BF16, kind="ExternalInput")
    wv_d = nc.dram_tensor("wv", [2, 128, D], BF16, kind="ExternalInput")
    bq_d = nc.dram_tensor("bq", [2, 128, 1], F32, kind="ExternalInput")
    bk_d = nc.dram_tensor("bk", [2, 128, 1], F32, kind="ExternalInput")
    bvb_d = nc.dram_tensor("bvb", [128, D], F32, kind="ExternalInput")
    mask_d = nc.dram_tensor("mask", [128, 256], F32, kind="ExternalInput")
    y_d = nc.dram_tensor("y", [tq, D], F32, kind="ExternalOutput")

    with tile.TileContext(nc) as tc:
        with (
            tc.tile_pool(name="persist", bufs=1) as pp,
            tc.tile_pool(name="vpool", bufs=1) as vp,
            tc.tile_pool(name="pj_ps", bufs=2, space="PSUM") as pj_ps,
            tc.tile_pool(name="st_ps", bufs=3, space="PSUM") as st_ps,
            tc.tile_pool(name="o_ps", bufs=2, space="PSUM") as o_ps_pool,
            tc.tile_pool(name="ptp", bufs=4) as ptp,
            tc.tile_pool(name="outp", bufs=3) as outp,
            tc.tile_pool(name="finp", bufs=3) as finp,
        ):
            # ---- persistent SBUF inputs
            xT = [pp.tile([128, t], BF16, name=f"xT{k}") for k in range(2)]
            xqT = [pp.tile([128, tq], BF16, name=f"xqT{k}") for k in range(2)]
            wq = [pp.tile([128, D], BF16, name=f"wq{k}") for k in range(2)]
            wk = [pp.tile([128, D], BF16, name=f"wk{k}") for k in range(2)]
            wv = [pp.tile([128, D], BF16, name=f"wv{k}") for k in range(2)]
            bq = [pp.tile([128, 1], F32, name=f"bq{k}") for k in range(2)]
            bk = [pp.tile([128, 1], F32, name=f"bk{k}") for k in range(2)]
            bvb = pp.tile([128, D], F32, name="bvb")
            mask = pp.tile([128, 256], F32, name="mask")
            for k in range(2):
                nc.sync.dma_start(xT[k][:], xT_d[k])
                nc.sync.dma_start(xqT[k][:], xqT_d[k])
                nc.sync.dma_start(wq[k][:], wq_d[k])
                nc.sync.dma_start(wk[k][:], wk_d[k])
                nc.sync.dma_start(wv[k][:], wv_d[k])
                nc.sync.dma_start(bq[k][:], bq_d[k])
                nc.sync.dma_start(bk[k][:], bk_d[k])
            nc.sync.dma_start(bvb[:], bvb_d[:])
            nc.sync.dma_start(mask[:], mask_d[:])

            # ---- projections: KT/QT in [dout, t] layout (bias via DVE)
            KT = [pp.tile([128, t], BF16, name=f"KT{m}") for m in range(2)]
            QT = [pp.tile([128, tq], BF16, name=f"QT{m}") for m in range(2)]
            NBK = min(512, t)
            NBQ = min(512, tq)
            for m in range(2):
                ms = slice(m * 128, (m + 1) * 128)
                for nb in range(t // NBK):
                    ps = pj_ps.tile([128, NBK], F32, name="pj", tag="pj")
                    for k in range(2):
                        nc.tensor.matmul(
                            ps[:],
                            wk[k][:, ms],
                            xT[k][:, nb * NBK : (nb + 1) * NBK],
                            start=(k == 0),
                            stop=(k == 1),
                        )
                    nc.vector.tensor_scalar_add(
                        KT[m][:, nb * NBK : (nb + 1) * NBK], ps[:], bk[m][:]
                    )
                for nb in range(tq // NBQ):
                    ps = pj_ps.tile([128, NBQ], F32, name="pj", tag="pj")
                    for k in range(2):
                        nc.tensor.matmul(
                            ps[:],
                            wq[k][:, ms],
                            xqT[k][:, nb * NBQ : (nb + 1) * NBQ],
                            start=(k == 0),
                            stop=(k == 1),
                        )
                    nc.vector.tensor_scalar_add(
                        QT[m][:, nb * NBQ : (nb + 1) * NBQ], ps[:], bq[m][:]
                    )

            # ---- V projection: natural [s, d] layout + ones column
            V = [vp.tile([128, D + 1], BF16, name=f"v{s}") for s in range(ns)]
            for s in range(ns):
                ps = pj_ps.tile([128, D], F32, name="pj", tag="pj")
                for k in range(2):
                    nc.tensor.matmul(
                        ps[:],
                        xT[k][:, s * 128 : (s + 1) * 128],
                        wv[k][:],
                        start=(k == 0),
                        stop=(k == 1),
                    )
                nc.vector.tensor_add(V[s][:, 0:D], ps[:], bvb[:])
                nc.vector.memset(V[s][:, D : D + 1], 1.0)

            # ---- attention
            exp_t = mybir.ActivationFunctionType.Exp
            for i in range(nq):
                e = 2 * i + 2  # s-tiles this query tile touches
                o_ps = o_ps_pool.tile([128, D + 1], F32, name="ops", tag="ops")
                qs = slice(i * 128, (i + 1) * 128)
                for s in range(e):
                    stp = st_ps.tile([128, 128], F32, name="stp", tag="stp")
                    for k in range(2):
                        nc.tensor.matmul(
                            stp[:],
                            KT[k][:, s * 128 : (s + 1) * 128],
                            QT[k][:, qs],
                            start=(k == 0),
                            stop=(k == 1),
                        )
                    if s == e - 2:
                        nc.vector.tensor_add(stp[:], stp[:], mask[:, 0:128])
                    elif s == e - 1:
                        nc.vector.tensor_add(stp[:], stp[:], mask[:, 128:256])
                    pt = ptp.tile([128, 128], BF16, name="pt", tag="pt")
                    nc.scalar.activation(pt[:], stp[:], exp_t, scale=scale)
                    nc.tensor.matmul(
                        o_ps[:], pt[:], V[s][:], start=(s == 0), stop=(s == e - 1)
                    )
                rec = finp.tile([128, 1], F32, name="rec", tag="rec")
                nc.vector.reciprocal(rec[:], o_ps[:, D : D + 1])
                ob = outp.tile([128, D], F32, name="ob", tag="ob")
                nc.vector.tensor_scalar_mul(ob[:], o_ps[:, 0:D], rec[:])
                nc.sync.dma_start(y_d[i * 128 : (i + 1) * 128, :], ob[:])
    return nc


def build_nc_v2(t: int = T, tq: int = TQ) -> bass.Bass:
    """Quad-grouped attention: 4 query tiles (512 q cols) share each S^T
    matmul / exp pass. Core h owns global q-tiles {8g + 2u + h}; group g
    runs a uniform s-extent of 8g+8 tiles on every core."""
    nq = tq // 128
    ns = t // 128
    ng = nq // 4
    assert t == 2 * tq and nq % 4 == 0
    scale = 1.0 / float(np.sqrt(np.float32(D)))

    nc = bacc.Bacc()
    xT_d = nc.dram_tensor("xT", [2, 128, t], BF16, kind="ExternalInput")
    xqT_d = nc.dram_tensor("xqT", [2, 128, tq], BF16, kind="ExternalInput")
    wq_d = nc.dram_tensor("wq", [2, 128, D], BF16, kind="ExternalInput")
    wk_d = nc.dram_tensor("wk", [2, 128, D], BF16, kind="ExternalInput")
    wv_d = nc.dram_tensor("wv", [2, 128, D], BF16, kind="ExternalInput")
    bq_d = nc.dram_tensor("bq", [2, 128, 1], F32, kind="ExternalInput")
    bk_d = nc.dram_tensor("bk", [2, 128, 1], F32, kind="ExternalInput")
    bvb_d = nc.dram_tensor("bvb", [128, D], F32, kind="ExternalInput")
    mask_d = nc.dram_tensor("mask", [8, 128, 512], F32, kind="ExternalInput")
    y_d = nc.dram_tensor("y", [tq, D], F32, kind="ExternalOutput")

    with tile.TileContext(nc) as tc:
        with (
            tc.tile_pool(name="persist", bufs=1) as pp,
            tc.tile_pool(name="vpool", bufs=1) as vp,
            tc.tile_pool(name="st_ps", bufs=2, space="PSUM") as st_ps,
            tc.tile_pool(name="o_ps", bufs=1, space="PSUM") as o_ps_pool,
            tc.tile_pool(name="ptp", bufs=3) as ptp,
            tc.tile_pool(name="outp", bufs=3) as outp,
            tc.tile_pool(name="finp", bufs=3) as finp,
        ):
            # ---- persistent SBUF inputs
            xT = [pp.tile([128, t], BF16, name=f"xT{k}") for k in range(2)]
            xqT = [pp.tile([128, tq], BF16, name=f"xqT{k}") for k in range(2)]
            wq = [pp.tile([128, D], BF16, name=f"wq{k}") for k in range(2)]
            wk = [pp.tile([128, D], BF16, name=f"wk{k}") for k in range(2)]
            wv = [pp.tile([128, D], BF16, name=f"wv{k}") for k in range(2)]
            bq = [pp.tile([128, 1], F32, name=f"bq{k}") for k in range(2)]
            bk = [pp.tile([128, 1], F32, name=f"bk{k}") for k in range(2)]
            bvb = pp.tile([128, D], F32, name="bvb")
            mask = [pp.tile([128, 512], F32, name=f"mask{r}") for r in range(8)]
            for k in range(2):
                nc.sync.dma_start(xT[k][:], xT_d[k])
                nc.sync.dma_start(xqT[k][:], xqT_d[k])
                nc.sync.dma_start(wq[k][:], wq_d[k])
                nc.sync.dma_start(wk[k][:], wk_d[k])
                nc.sync.dma_start(wv[k][:], wv_d[k])
                nc.sync.dma_start(bq[k][:], bq_d[k])
                nc.sync.dma_start(bk[k][:], bk_d[k])
            nc.sync.dma_start(bvb[:], bvb_d[:])
            for r in range(8):
                nc.sync.dma_start(mask[r][:], mask_d[r])

            KT = [pp.tile([128, t], BF16, name=f"KT{m}") for m in range(2)]
            QT = [pp.tile([128, tq], BF16, name=f"QT{m}") for m in range(2)]
            V = [vp.tile([128, D + 1], BF16, name=f"v{s}") for s in range(ns)]

            # ---- projections in their own PSUM pool (freed before attention)
            with tc.tile_pool(name="pj_ps", bufs=2, space="PSUM") as pj_ps:
                NBK = min(512, t)
                NBQ = min(512, tq)
                for m in range(2):
                    ms = slice(m * 128, (m + 1) * 128)
                    for nb in range(t // NBK):
                        ps = pj_ps.tile([128, NBK], F32, name="pj", tag="pj")
                        for k in range(2):
                            nc.tensor.matmul(
                                ps[:],
                                wk[k][:, ms],
                                xT[k][:, nb * NBK : (nb + 1) * NBK],
                                start=(k == 0),
                                stop=(k == 1),
                            )
                        nc.vector.tensor_scalar_add(
                            KT[m][:, nb * NBK : (nb + 1) * NBK], ps[:], bk[m][:]
                        )
                    for nb in range(tq // NBQ):
                        ps = pj_ps.tile([128, NBQ], F32, name="pj", tag="pj")
                        for k in range(2):
                            nc.tensor.matmul(
                                ps[:],
                                wq[k][:, ms],
                                xqT[k][:, nb * NBQ : (nb + 1) * NBQ],
                                start=(k == 0),
                                stop=(k == 1),
                            )
                        nc.vector.tensor_scalar_add(
                            QT[m][:, nb * NBQ : (nb + 1) * NBQ], ps[:], bq[m][:]
                        )
                for s in range(ns):
                    ps = pj_ps.tile([128, D], F32, name="pj", tag="pj")
                    for k in range(2):
                        nc.tensor.matmul(
                            ps[:],
                            xT[k][:, s * 128 : (s + 1) * 128],
                            wv[k][:],
                            start=(k == 0),
                            stop=(k == 1),
                        )
                    nc.vector.tensor_add(V[s][:, 0:D], ps[:], bvb[:])
                    nc.vector.memset(V[s][:, D : D + 1], 1.0)

            # ---- attention, 512 q cols per group
            exp_t = mybir.ActivationFunctionType.Exp
            att = ctx_att = tc.tile_pool(name="st_ps", bufs=2, space="PSUM")
            st_ps = att.__enter__()
            o_ctx = tc.tile_pool(name="o_ps", bufs=6, space="PSUM")
            o_ps_pool = o_ctx.__enter__()
            for g in range(ng):
                e = 8 * g + 8
                qs = slice(g * 512, (g + 1) * 512)
                o = [
                    o_ps_pool.tile([128, D + 1], F32, name=f"o{u}", tag=f"o{u}")
                    for u in range(4)
                ]
                for s in range(e):
                    stp = st_ps.tile([128, 512], F32, name="stp", tag="stp")
                    for k in range(2):
                        nc.tensor.matmul(
                            stp[:],
                            KT[k][:, s * 128 : (s + 1) * 128],
                            QT[k][:, qs],
                            start=(k == 0),
                            stop=(k == 1),
                        )
                    if s >= 8 * g:
                        nc.vector.tensor_add(stp[:], stp[:], mask[s - 8 * g][:])
                    pt = ptp.tile([128, 512], BF16, name="pt", tag="pt")
                    nc.scalar.activation(pt[:], stp[:], exp_t, scale=scale)
                    for u in range(4):
                        nc.tensor.matmul(
                            o[u][:],
                            pt[:, u * 128 : (u + 1) * 128],
                            V[s][:],
                            start=(s == 0),
                            stop=(s == e - 1),
                        )
                for u in range(4):
                    rec = finp.tile([128, 1], F32, name="rec", tag="rec")
                    nc.vector.reciprocal(rec[:], o[u][:, D : D + 1])
                    ob = outp.tile([128, D], F32, name="ob", tag="ob")
                    nc.vector.tensor_scalar_mul(ob[:], o[u][:, 0:D], rec[:])
                    lrow = (g * 4 + u) * 128
                    nc.sync.dma_start(y_d[lrow : lrow + 128, :], ob[:])
    return nc


def build_nc_v3(t: int = T, tq: int = TQ, st_bufs: int = 4, o_bufs: int = 4, pt_bufs: int = 6, pj_bufs: int = 4) -> bass.Bass:
    """v2 + cheaper masking, less dead work, and walrus-friendly syncs:
    - all constants (weights, biases, masks) packed into two DRAM tensors
      loaded with one DMA each; tiny DVE "absorber" copies pull the DMA
      completion into DVE's vector clock so the bias TensorScalarPtr ops
      carry a single sem wait (walrus rejects multi-wait TS instrs);
    - causal mask applied AFTER exp as a multiplicative 0/1 bf16 mask on
      one 128-col block per diagonal s-tile (DVE bf16 SBUF fast mode);
    - PV matmuls skipped for (s_rel, u) tiles dead on BOTH cores
      (u < floor(s_rel/2)) — the skip pattern is SPMD-uniform;
    - input x DMAs chunked so projections overlap the loads;
    - single-tag o-pool (bufs=6) so group g+1 does not wait on group g's
      finalize."""
    nq = tq // 128
    ns = t // 128
    ng = nq // 4
    assert t == 2 * tq and nq % 4 == 0
    scale = 1.0 / float(np.sqrt(np.float32(D)))

    nc = bacc.Bacc()
    xT_d = nc.dram_tensor("xT", [2, 128, t], BF16, kind="ExternalInput")
    xqT_d = nc.dram_tensor("xqT", [2, 128, tq], BF16, kind="ExternalInput")
    cw_d = nc.dram_tensor("cw", [128, 1536], BF16, kind="ExternalInput")
    cm_d = nc.dram_tensor("cm", [128, 1024], BF16, kind="ExternalInput")
    cf_d = nc.dram_tensor("cf", [128, 260], F32, kind="ExternalInput")
    y_d = nc.dram_tensor("y", [tq, D], F32, kind="ExternalOutput")

    with tile.TileContext(nc) as tc:
        with (
            tc.tile_pool(name="persist", bufs=1) as pp,
            tc.tile_pool(name="vpool", bufs=1) as vp,
            tc.tile_pool(name="ptp", bufs=pt_bufs) as ptp,
            tc.tile_pool(name="outp", bufs=3) as outp,
            tc.tile_pool(name="finp", bufs=4) as finp,
        ):
            # ---- inputs. One sync (HWDGE) queue so transfers complete in
            # priority order: weights -> first x chunks (gates the first
            # projection matmuls) -> rest -> masks (needed ~20us in).
            cw = pp.tile([128, 1536], BF16, name="cw")
            cm = pp.tile([128, 1024], BF16, name="cm")
            cf = pp.tile([128, 260], F32, name="cf")
            xT = [pp.tile([128, t], BF16, name=f"xT{k}") for k in range(2)]
            xqT = [pp.tile([128, tq], BF16, name=f"xqT{k}") for k in range(2)]
            CH = max(512, t // 2)
            nc.sync.dma_start(cw[:], cw_d[:])
            for k in range(2):
                nc.sync.dma_start(xT[k][:, 0:CH], xT_d[k][:, 0:CH])
            nc.sync.dma_start(cf[:], cf_d[:])
            for c0 in range(CH, t, CH):
                for k in range(2):
                    nc.sync.dma_start(xT[k][:, c0 : c0 + CH], xT_d[k][:, c0 : c0 + CH])
            for k in range(2):
                nc.sync.dma_start(xqT[k][:], xqT_d[k])
            nc.sync.dma_start(cm[:], cm_d[:])
            # absorber copies: pull each const DMA's completion into DVE's
            # vector clock so downstream DVE ops carry a single sem wait
            scrb = finp.tile([128, 1], BF16, name="scrb", tag="scrb")
            nc.vector.tensor_copy(scrb[:], cw[:, 0:1])
            scrf = finp.tile([128, 1], F32, name="scrf", tag="scrf")
            nc.vector.tensor_copy(scrf[:], cf[:, 0:1])
            scrm = finp.tile([128, 1], BF16, name="scrm", tag="scrm")
            nc.vector.tensor_copy(scrm[:], cm[:, 0:1])
            wq = [cw[:, 0 + k * 256 : 256 + k * 256] for k in range(2)]
            wk = [cw[:, 512 + k * 256 : 768 + k * 256] for k in range(2)]
            wv = [cw[:, 1024 + k * 256 : 1280 + k * 256] for k in range(2)]
            maskb = [cm[:, r * 128 : (r + 1) * 128] for r in range(8)]
            bq = [cf[:, k : k + 1] for k in range(2)]
            bk = [cf[:, 2 + k : 3 + k] for k in range(2)]
            bvb = cf[:, 4:260]

            # HAM warm-up: garbage matmuls while input DMAs land, so the
            # PE clock gate is already at 8/8 when real work arrives.
            wa = pp.tile([128, 128], BF16, name="wa")
            wb = pp.tile([128, 512], BF16, name="wb")
            nc.vector.memset(wa[:], 0.0)
            nc.vector.memset(wb[:], 0.0)
            with tc.tile_pool(name="warm_ps", bufs=1, space="PSUM") as wps:
                wp_t = wps.tile([128, 512], F32, name="warm")
                for _ in range(20):
                    nc.tensor.matmul(wp_t[:], wa[:], wb[:], start=True, stop=True)

            KT = [pp.tile([128, t], BF16, name=f"KT{m}") for m in range(2)]
            QT = [pp.tile([128, tq], BF16, name=f"QT{m}") for m in range(2)]
            V = [vp.tile([128, D + 1], BF16, name=f"v{s}") for s in range(ns)]

            # ---- projections in their own PSUM pool (freed before attention)
            with tc.tile_pool(name="pj_ps", bufs=pj_bufs, space="PSUM") as pj_ps:
                NBK = min(512, t)
                NBQ = min(512, tq)
                for nb in range(t // NBK):
                    for m in range(2):
                        ms = slice(m * 128, (m + 1) * 128)
                        ps = pj_ps.tile([128, NBK], F32, name="pj", tag="pj")
                        for k in range(2):
                            nc.tensor.matmul(
                                ps[:],
                                wk[k][:, ms],
                                xT[k][:, nb * NBK : (nb + 1) * NBK],
                                start=(k == 0),
                                stop=(k == 1),
                            )
                        nc.vector.tensor_scalar_add(
                            KT[m][:, nb * NBK : (nb + 1) * NBK], ps[:], bk[m]
                        )
                for m in range(2):
                    ms = slice(m * 128, (m + 1) * 128)
                    for nb in range(tq // NBQ):
                        ps = pj_ps.tile([128, NBQ], F32, name="pj", tag="pj")
                        for k in range(2):
                            nc.tensor.matmul(
                                ps[:],
                                wq[k][:, ms],
                                xqT[k][:, nb * NBQ : (nb + 1) * NBQ],
                                start=(k == 0),
                                stop=(k == 1),
                            )
                        nc.vector.tensor_scalar_add(
                            QT[m][:, nb * NBQ : (nb + 1) * NBQ], ps[:], bq[m]
                        )

            # ---- attention, 512 q cols per group
            exp_t = mybir.ActivationFunctionType.Exp
            att = ctx_att = tc.tile_pool(name="st_ps", bufs=st_bufs, space="PSUM")
            st_ps = att.__enter__()
            o_ctx = tc.tile_pool(name="o_ps", bufs=o_bufs, space="PSUM")
            o_ps_pool = o_ctx.__enter__()
            for g in range(ng):
                e = 8 * g + 8
                for s in range(8 * g, min(8 * g + 8, ns)):
                    ps = st_ps.tile([128, D], F32, name="vpj", tag="stp")
                    for k in range(2):
                        nc.tensor.matmul(
                            ps[:],
                            xT[k][:, s * 128 : (s + 1) * 128],
                            wv[k],
                            start=(k == 0),
                            stop=(k == 1),
                        )
                    # bias-free V: since sum_s P = den, (O + den*bv)/den =
                    # O/den + bv, so bv moves to the finalize and this
                    # PSUM->SBUF cast-copy runs on the idle ACT engine
                    nc.scalar.copy(V[s][:, 0:D], ps[:])
                    nc.vector.memset(V[s][:, D : D + 1], 1.0)
                qs = slice(g * 512, (g + 1) * 512)
                o = [
                    o_ps_pool.tile([128, D + 1], F32, name=f"o{u}", tag="o")
                    for u in range(4)
                ]
                for s in range(e):
                    s_rel = s - 8 * g
                    u0 = max(s_rel, 0) // 2  # first live 128-col block
                    c0 = u0 * 128
                    stp = st_ps.tile([128, 512], F32, name="stp", tag="stp")
                    for k in range(2):
                        nc.tensor.matmul(
                            stp[:, c0:512],
                            KT[k][:, s * 128 : (s + 1) * 128],
                            QT[k][:, g * 512 + c0 : (g + 1) * 512],
                            start=(k == 0),
                            stop=(k == 1),
                        )
                    pt = ptp.tile([128, 512], BF16, name="pt", tag="pt")
                    nc.scalar.activation(
                        pt[:, c0:512], stp[:, c0:512], exp_t, scale=scale
                    )
                    if s_rel >= 0:
                        nc.vector.tensor_mul(
                            pt[:, c0 : c0 + 128],
                            pt[:, c0 : c0 + 128],
                            maskb[s_rel],
                        )
                    for u in range(4):
                        if s_rel >= 0 and u < s_rel // 2:
                            continue  # dead on every core
                        nc.tensor.matmul(
                            o[u][:],
                            pt[:, u * 128 : (u + 1) * 128],
                            V[s][:],
                            start=(s == 0),
                            stop=(s == 8 * g + 2 * u + 1),
                        )
                for u in range(4):
                    rec = finp.tile([128, 1], F32, name="rec", tag="rec")
                    nc.vector.reciprocal(rec[:], o[u][:, D : D + 1])
                    ob = outp.tile([128, D], F32, name="ob", tag="ob")
                    nc.vector.scalar_tensor_tensor(
                        ob[:],
                        o[u][:, 0:D],
                        rec[:],
                        bvb,
                        mybir.AluOpType.mult,
                        mybir.AluOpType.add,
                    )
                    lrow = (g * 4 + u) * 128
                    nc.sync.dma_start(y_d[lrow : lrow + 128, :], ob[:])
            o_ctx.__exit__(None, None, None)
            ctx_att.__exit__(None, None, None)
    return nc


def build_nc_v4(
    t: int = T,
    tq: int = TQ,
    st_bufs: int = 3,
    o_bufs: int = 5,
    pt_bufs: int = 6,
    warm_n: int = 12,
) -> bass.Bass:
    """v3 + fp8 S^T and a fully interleaved projection/attention pipeline.

    - S^T = K^T·Q runs as ONE fp8e4 DoubleRow matmul per (s, group): the PE
      contracts all 256 d-rows in a single pass (2 rows/cycle), halving the
      S cost vs the bf16 2-pass version. Q/K are projected in bf16 precision
      (PSUM f32) and only quantized at the PSUM->SBUF cast (measured rel err
      ~1.1e-2 vs fp32 reference, tolerance 2e-2). P and V stay bf16 (fp8
      there costs ~3.6% rms -> fails tolerance).
    - K's bias is dropped entirely: softmax is invariant to the row-constant
      q·bk term, so only bq (via the Q cast) matters.
    - Projections are interleaved per query-group g and allocate their PSUM
      from the SAME pool/tag as the o accumulators (every slot is a full
      bank anyway): the 4-deep rotation hides the ~0.7us cast latency that
      a dedicated 2-buf pool exposed, and st_ps gets 4 banks for deeper S
      lookahead. Casts/copies alternate DVE/ACT to split the backlog.
    - Inputs stream on BOTH hardware DGE queues (SP + ACT); transfers on one
      queue serialize (each push waits the previous transfer's completion
      semaphore), so the host packs x/xq/consts into few, need-ordered
      transfers: [cwm | xq-g0 | cf | xq-rest] on ACT, 4 x group-chunks on SP.
    - o[u] finalize (reciprocal + scale + bias + output DMA) is emitted
      inside the s-loop right after u's stopping matmul, so the tail after
      the last PV matmul is just one finalize chain.
    """
    nq = tq // 128
    ns = t // 128
    ng = nq // 4
    assert t == 2 * tq and nq % 4 == 0
    scale = 1.0 / float(np.sqrt(np.float32(D)))
    FP8 = mybir.dt.float8e4

    nc = bacc.Bacc()
    xT_d = nc.dram_tensor("xT", [128, 2, t], BF16, kind="ExternalInput")
    xqT_d = nc.dram_tensor("xqT", [128, 2, tq], BF16, kind="ExternalInput")
    cwm_d = nc.dram_tensor("cwm", [128, 2560], BF16, kind="ExternalInput")
    cf_d = nc.dram_tensor("cf", [128, 258], F32, kind="ExternalInput")
    y_d = nc.dram_tensor("y", [tq, D], F32, kind="ExternalOutput")

    with tile.TileContext(nc) as tc:
        with (
            tc.tile_pool(name="persist", bufs=1) as pp,
            tc.tile_pool(name="vpool", bufs=1) as vp,
            tc.tile_pool(name="ptp", bufs=pt_bufs) as ptp,
            tc.tile_pool(name="outp", bufs=3) as outp,
            tc.tile_pool(name="finp", bufs=4) as finp,
        ):
            cwm = pp.tile([128, 2560], BF16, name="cwm")
            cf = pp.tile([128, 258], F32, name="cf")
            xTa = pp.tile([128, 2 * t], BF16, name="xTa")
            xqTa = pp.tile([128, 2 * tq], BF16, name="xqTa")
            xTv = xTa[:].rearrange("p (k c) -> p k c", k=2)
            xqTv = xqTa[:].rearrange("p (k c) -> p k c", k=2)
            xT = [xTa[:, k * t : (k + 1) * t] for k in range(2)]
            xqT = [xqTa[:, k * tq : (k + 1) * tq] for k in range(2)]

            def xts(k, a, b):
                return xT[k][:, a:b]

            def xqs(k, a, b):
                return xqT[k][:, a:b]
            KT8 = pp.tile([128, 2 * t], FP8, name="KT8")
            QT8 = pp.tile([128, 2 * tq], FP8, name="QT8")
            KT8v = KT8[:].rearrange("p (m c) -> p m c", m=2)
            QT8v = QT8[:].rearrange("p (m c) -> p m c", m=2)
            V = [vp.tile([128, D + 1], BF16, name=f"v{s}") for s in range(ns)]

            # ---- input DMA: the two DGE queues share ~358GB/s of HBM and
            # the ACT-side queue starts late and runs at ~half rate, so the
            # group-0-critical set streams need-ordered on the fast SP
            # queue; only the latest-needed bulk rides the ACT queue.
            nc.scalar.dma_start(xqTv[:, :, 0:512], xqT_d[:, :, 0:512])
            nc.scalar.dma_start(cf[:], cf_d[:])
            nc.sync.dma_start(cwm[:], cwm_d[:])
            nc.sync.dma_start(xTv[:, :, 0:1024], xT_d[:, :, 0:1024])
            nc.sync.dma_start(xTv[:, :, 1024:2048], xT_d[:, :, 1024:2048])
            nc.sync.dma_start(xqTv[:, :, 512:1024], xqT_d[:, :, 512:1024])
            nc.sync.dma_start(xTv[:, :, 2048:3072], xT_d[:, :, 2048:3072])
            nc.sync.dma_start(xqTv[:, :, 1024:tq], xqT_d[:, :, 1024:tq])
            nc.sync.dma_start(xTv[:, :, 3072:t], xT_d[:, :, 3072:t])

            wq = [cwm[:, 0 + k * 256 : 256 + k * 256] for k in range(2)]
            wk = [cwm[:, 512 + k * 256 : 768 + k * 256] for k in range(2)]
            wv = [cwm[:, 1024 + k * 256 : 1280 + k * 256] for k in range(2)]
            maskb = [cwm[:, 1536 + r * 128 : 1536 + (r + 1) * 128] for r in range(8)]
            bq = [cf[:, k : k + 1] for k in range(2)]
            bvb = cf[:, 2:258]

            # warm-up garbage matmuls bridge the input-DMA wait so the PE
            # HAM clock gate is at 8/8 when real work arrives
            wa = pp.tile([128, 128], BF16, name="wa")
            wb = pp.tile([128, 512], BF16, name="wb")
            nc.vector.memset(wa[:], 0.0)
            nc.vector.memset(wb[:], 0.0)
            with tc.tile_pool(name="warm_ps", bufs=1, space="PSUM") as wps:
                wp_t = wps.tile([128, 512], F32, name="warm")
                for _ in range(warm_n):
                    nc.tensor.matmul(wp_t[:], wa[:], wb[:], start=True, stop=True)

            # denominator ones-column, set once per V tile (off critical path)
            for s in range(ns):
                nc.vector.memset(V[s][:, D : D + 1], 1.0)
            # absorber copies pull const DMA completions into each consumer
            # engine's vector clock (single extra sem wait per consumer op)
            scrb = finp.tile([128, 1], BF16, name="scrb", tag="scrb")
            nc.vector.tensor_copy(scrb[:], cwm[:, 0:1])
            scrf = finp.tile([128, 1], F32, name="scrf", tag="scrf")
            nc.vector.tensor_copy(scrf[:], cf[:, 0:1])
            scrg = finp.tile([128, 1], F32, name="scrg", tag="scrg")
            nc.scalar.copy(scrg[:], cf[:, 0:1])

            exp_t = mybir.ActivationFunctionType.Exp
            id_t = mybir.ActivationFunctionType.Identity
            with (
                tc.tile_pool(name="st_ps", bufs=st_bufs, space="PSUM") as st_ps,
                tc.tile_pool(name="o_ps", bufs=o_bufs, space="PSUM") as o_ps_pool,
            ):
                def q_proj(g):
                    for m in range(2):
                        ms = slice(m * 128, (m + 1) * 128)
                        ps = o_ps_pool.tile([128, 512], F32, name="pj", tag="o")
                        for k in range(2):
                            nc.tensor.matmul(
                                ps[:], wq[k][:, ms],
                                xqs(k, g * 512, (g + 1) * 512),
                                start=(k == 0), stop=(k == 1),
                            )
                        dst = QT8[:, m * tq + g * 512 : m * tq + (g + 1) * 512]
                        if m == 0:
                            nc.vector.tensor_scalar_add(dst, ps[:], bq[m])
                        else:
                            nc.scalar.activation(dst, ps[:], id_t, bias=bq[m])

                def k_proj(g):
                    # no bias: q·bk is row-constant, softmax-invariant
                    for nb in range(2):
                        ca = g * 1024 + nb * 512
                        for m in range(2):
                            ms = slice(m * 128, (m + 1) * 128)
                            ps = o_ps_pool.tile([128, 512], F32, name="pj", tag="o")
                            for k in range(2):
                                nc.tensor.matmul(
                                    ps[:], wk[k][:, ms], xts(k, ca, ca + 512),
                                    start=(k == 0), stop=(k == 1),
                                )
                            dst = KT8[
                                :,
                                m * t + g * 1024 + nb * 512 :
                                m * t + g * 1024 + (nb + 1) * 512,
                            ]
                            if (2 * nb + m) % 2 == 0:
                                nc.vector.tensor_copy(dst, ps[:])
                            else:
                                nc.scalar.copy(dst, ps[:])

                def v_proj(g):
                    # bias-free; bv is added in the finalize
                    for s in range(8 * g, 8 * g + 8):
                        ps = o_ps_pool.tile([128, D], F32, name="vpj", tag="o")
                        for k in range(2):
                            nc.tensor.matmul(
                                ps[:], xts(k, s * 128, (s + 1) * 128), wv[k],
                                start=(k == 0), stop=(k == 1),
                            )
                        if s % 2 == 0:
                            nc.scalar.copy(V[s][:, 0:D], ps[:])
                        else:
                            nc.vector.tensor_copy(V[s][:, 0:D], ps[:])

                for g in range(ng):
                    e = 8 * g + 8
                    q_proj(g), k_proj(g), v_proj(g)
                    if g == 0:
                        scrm = finp.tile([128, 1], BF16, name="scrm", tag="scrm")
                        nc.vector.tensor_copy(scrm[:], cwm[:, 1536:1537])
                    # ---- attention for group g
                    o = [
                        o_ps_pool.tile([128, D + 1], F32, name=f"o{u}", tag="o")
                        for u in range(4)
                    ]
                    for s in range(e):
                        s_rel = s - 8 * g
                        u0 = max(s_rel, 0) // 2
                        c0 = u0 * 128
                        stp = st_ps.tile([128, 512], F32, name="stp", tag="stp")
                        nc.tensor.matmul(
                            stp[:, c0:512],
                            KT8v[:, :, s * 128 : (s + 1) * 128],
                            QT8v[:, :, g * 512 + c0 : (g + 1) * 512],
                            start=True, stop=True,
                            perf_mode=mybir.MatmulPerfMode.DoubleRow,
                        )
                        pt = ptp.tile([128, 512], BF16, name="pt", tag="pt")
                        nc.scalar.activation(
                            pt[:, c0:512], stp[:, c0:512], exp_t, scale=scale
                        )
                        if s_rel >= 0:
                            nc.vector.tensor_mul(
                                pt[:, c0 : c0 + 128],
                                pt[:, c0 : c0 + 128],
                                maskb[s_rel],
                            )
                        # u0's P block waits on the DVE mask -> run it last
                        for u in list(range(u0 + 1, 4)) + [u0]:
                            nc.tensor.matmul(
                                o[u][:],
                                pt[:, u * 128 : (u + 1) * 128],
                                V[s][:],
                                start=(s == 0),
                                stop=(s == 8 * g + 2 * u + 1),
                            )
                            if s == 8 * g + 2 * u + 1:
                                rec = finp.tile([128, 1], F32, name="rec", tag="rec")
                                nc.vector.reciprocal(rec[:], o[u][:, D : D + 1])
                                ob = outp.tile([128, D], F32, name="ob", tag="ob")
                                nc.vector.scalar_tensor_tensor(
                                    ob[:], o[u][:, 0:D], rec[:], bvb,
                                    mybir.AluOpType.mult, mybir.AluOpType.add,
                                )
                                lrow = (g * 4 + u) * 128
                                nc.sync.dma_start(y_d[lrow : lrow + 128, :], ob[:])
    return nc


def prep_inputs(
    x, Wq, bq, Wk, bk, Wv, bv, t: int = T, n_cores: int = N_CORES, version: int = 1
):
    """Per-core input maps (host-side shard / transpose / cast)."""
    x = np.asarray(x, dtype=np.float32)
    b_dim = x.shape[0]
    tq = t // 2
    nq = tq // 128
    shared = {}
    for name, w in (("wq", Wq), ("wk", Wk), ("wv", Wv)):
        shared[name] = np.ascontiguousarray(
            np.asarray(w, np.float32).astype(NPBF16).reshape(2, 128, D)
        )
    shared["bq"] = np.ascontiguousarray(
        np.asarray(bq, np.float32).reshape(2, 128, 1)
    )
    shared["bk"] = np.ascontiguousarray(
        np.asarray(bk, np.float32).reshape(2, 128, 1)
    )
    shared["bvb"] = np.ascontiguousarray(
        np.broadcast_to(np.asarray(bv, np.float32), (128, D))
    )
    idx = np.arange(128)
    tri = np.where(idx[:, None] > idx[None, :], np.float32(NEG), np.float32(0.0))
    full = np.full((128, 128), NEG, np.float32)
    zero = np.zeros((128, 128), np.float32)
    if version == 1:
        masks = [
            np.ascontiguousarray(np.concatenate([tri, full], axis=1)),
            np.ascontiguousarray(np.concatenate([zero, tri], axis=1)),
        ]
    elif version == 2:
        masks = []
        for h in range(2):
            m = np.empty((8, 128, 512), np.float32)
            for s_rel in range(8):
                for u in range(4):
                    blk = full if s_rel > 2 * u + h else (tri if s_rel == 2 * u + h else zero)
                    m[s_rel, :, u * 128 : (u + 1) * 128] = blk
            masks.append(np.ascontiguousarray(m))
    else:
        # v3/v4: multiplicative 0/1 bf16 masks, one 128-block per diag s_rel.
        # s_rel even -> block u0=s_rel/2: h=0 diag (keep s<=q), h=1 keep-all
        # s_rel odd  -> block u0:         h=0 dead (zeros),     h=1 diag
        tri01 = (idx[:, None] <= idx[None, :]).astype(NPBF16)
        ones = np.ones((128, 128), NPBF16)
        zeros = np.zeros((128, 128), NPBF16)
        masks = []
        for h in range(2):
            m = np.empty((8, 128, 128), NPBF16)
            for s_rel in range(8):
                if s_rel % 2 == 0:
                    m[s_rel] = tri01 if h == 0 else ones
                else:
                    m[s_rel] = zeros if h == 0 else tri01
            masks.append(m)
        if version == 4:
            # v4: bk dropped (softmax-invariant) -> cf [128, 258] f32
            cf = np.empty((128, 258), np.float32)
            cf[:, 0:2] = np.asarray(bq, np.float32).reshape(2, 128).T
            cf[:, 2:258] = np.broadcast_to(np.asarray(bv, np.float32), (128, D))
        else:
            # pack constants: cw [128, 1536] bf16, cf [128, 260] f32
            cf = np.empty((128, 260), np.float32)
            cf[:, 0:2] = np.asarray(bq, np.float32).reshape(2, 128).T
            cf[:, 2:4] = np.asarray(bk, np.float32).reshape(2, 128).T
            cf[:, 4:260] = np.broadcast_to(np.asarray(bv, np.float32), (128, D))
        cw = np.empty((128, 1536), NPBF16)
        for j, w in enumerate((Wq, Wk, Wv)):
            wb = np.asarray(w, np.float32).astype(NPBF16).reshape(2, 128, D)
            cw[:, j * 512 : j * 512 + 256] = wb[0]
            cw[:, j * 512 + 256 : j * 512 + 512] = wb[1]
        cms = []
        for h in range(2):
            cm = np.empty((128, 1024), NPBF16)
            for r in range(8):
                cm[:, r * 128 : (r + 1) * 128] = masks[h][r]
            cms.append(np.ascontiguousarray(cm))
        if version == 4:
            # v4: weights+masks in one bf16 tensor; x/xq as [128, k, cols]
            cwms = [
                np.ascontiguousarray(np.concatenate([cw, cms[h]], axis=1))
                for h in range(2)
            ]
        shared = {"cf": np.ascontiguousarray(cf), "cw": np.ascontiguousarray(cw)}
    in_maps = []
    for c in range(n_cores):
        b, h = divmod(c, 2)
        xb = x[b % b_dim]  # [t, D]
        xT = np.ascontiguousarray(xb.T.astype(NPBF16).reshape(2, 128, t))
        qrows = np.concatenate(
            [xb[g * 128 : (g + 1) * 128] for g in _qtiles(nq, h, version)], axis=0
        )
        xqT = np.ascontiguousarray(qrows.T.astype(NPBF16).reshape(2, 128, tq))
        if version == 4:
            in_maps.append({
                "xT": np.ascontiguousarray(xT.transpose(1, 0, 2)),
                "xqT": np.ascontiguousarray(xqT.transpose(1, 0, 2)),
                "cwm": cwms[h],
                "cf": shared["cf"],
            })
        elif version == 3:
            in_maps.append({"xT": xT, "xqT": xqT, "cm": cms[h], **shared})
        else:
            in_maps.append({"xT": xT, "xqT": xqT, "mask": masks[h], **shared})
    return in_maps


def _qtiles(nq: int, h: int, version: int) -> list[int]:
    """Global q-tile index for each local tile, in local order."""
    if version == 1:
        return [2 * i + h for i in range(nq)]
    return [8 * g + 2 * u + h for g in range(nq // 4) for u in range(4)]


_BUILDERS = {1: build_nc, 2: build_nc_v2, 3: build_nc_v3, 4: build_nc_v4}


def gather_output(results, t: int = T, n_cores: int = N_CORES, version: int = 1):
    tq = t // 2
    nq = tq // 128
    y = np.empty((n_cores // 2, t, D), np.float32)
    for c in range(n_cores):
        b, h = divmod(c, 2)
        yc = np.asarray(results[c]["y"])
        for li, g in enumerate(_qtiles(nq, h, version)):
            y[b, g * 128 : (g + 1) * 128] = yc[li * 128 : (li + 1) * 128]
    return y


VERSION = 4


def run_on_hw(inputs: dict, trace: bool = False):
    """Returns (y [B,T,D] f32, BassKernelResults)."""
    in_maps = prep_inputs(**inputs, version=VERSION)
    nc = _BUILDERS[VERSION]()
    if not nc.is_finalized():
        nc.finalize()
    res = run_bass_kernel_spmd(nc, in_maps, list(range(N_CORES)), trace=trace)
    return gather_output(res.results, version=VERSION), res


def kernel(**inputs) -> np.ndarray:
    y, _ = run_on_hw(inputs, trace=False)
    return y



# revision 8
# speedup vs baseline: 1.0828x; 1.0828x over previous
"""Causal attention (B=4, T=4096, D=256) on 8 TRN2 NeuronCores.

Sharding: data-parallel over batch x query-halves. Core c handles batch
b = c//2 and query half h = c%2. The active builder (v3, VERSION=3)
groups queries 512 wide: group g of core h owns the interleaved global
128-row query tiles {8g + 2u + h : u in 0..3}, so both halves see the
same s-extent (8g+8 tiles) per group. That makes causal work exactly
balanced AND the program SPMD-uniform: every core runs the identical
instruction stream; only the input DATA (gathered query rows, per-core
0/1 mask blocks) differs.

On-chip layout (flash-attention style, nothing T^2-sized touches HBM):
  xT  [d, t]  (bf16)  -> QT/KT projections directly in transposed layout
                         (lhsT = W [din, dout], rhs = xT)
  S^T [s, q] = matmul(lhsT=KT_tile, rhs=QT_[512 q cols])  (contract d)
  P^T = exp(scale * S^T)  on ACT (no max-subtraction: logits are O(1),
        exp cannot overflow in fp32), then a multiplicative 0/1 bf16
        mask on the single diagonal 128-col block (DVE)
  O   [q, d] = sum_s matmul(lhsT=P^T 128-col slice, rhs=V_ext)
where V_ext has a ones column appended, so O[:, D] accumulates the
softmax denominator for free; final divide is a per-partition scalar.
S^T/exp are narrowed to live columns in the diagonal region and PV
matmuls that are dead on BOTH cores are skipped (both SPMD-uniform).
A short garbage-matmul warm-up bridges the input-DMA wait so the PE
HAM clock gate is at 8/8 when real work arrives, and the single sync
HWDGE queue streams inputs in consumption order (weights, first x
chunks, rest, masks last). V is stored bias-free (bv is added in the
finalize: O/den + bv == using V+bv, since the P rows sum to den), so
the V PSUM->SBUF cast-copies run on the otherwise-idle ACT engine.
Measured ~113-116 us on hardware (8 cores, run-to-run drift ~2 us),
rel err ~2.3e-3 vs the fp32 reference (bf16 matmul precision).
"""

import os
import sys

import numpy as np

for _p in ("/opt/trn_rl_repo", "/root/.axon_site/_ro/trn_rl_repo"):
    if os.path.isdir(_p) and _p not in sys.path:
        sys.path.insert(0, _p)

import ml_dtypes  # noqa: E402

import concourse.bass as bass  # noqa: E402
import concourse.bacc as bacc  # noqa: E402
import concourse.mybir as mybir  # noqa: E402
import concourse.tile as tile  # noqa: E402
from concourse.bass_utils import run_bass_kernel_spmd  # noqa: E402

BF16 = mybir.dt.bfloat16
F32 = mybir.dt.float32
NPBF16 = ml_dtypes.bfloat16

B = 4
T = 4096
D = 256
N_CORES = 8
TQ = T // 2  # query rows per core
NEG = -1.0e9


def build_nc(t: int = T, tq: int = TQ) -> bass.Bass:
    nq = tq // 128  # query tiles per core
    ns = t // 128  # total key tiles
    assert t == 2 * tq and ns == 2 * nq
    scale = 1.0 / float(np.sqrt(np.float32(D)))

    nc = bacc.Bacc()
    xT_d = nc.dram_tensor("xT", [2, 128, t], BF16, kind="ExternalInput")
    xqT_d = nc.dram_tensor("xqT", [2, 128, tq], BF16, kind="ExternalInput")
    wq_d = nc.dram_tensor("wq", [2, 128, D], BF16, kind="ExternalInput")
    wk_d = nc.dram_tensor("wk", [2, 128, D], BF16, kind="ExternalInput")
    wv_d = nc.dram_tensor("wv", [2, 128, D], BF16, kind="ExternalInput")
    bq_d = nc.dram_tensor("bq", [2, 128, 1], F32, kind="ExternalInput")
    bk_d = nc.dram_tensor("bk", [2, 128, 1], F32, kind="ExternalInput")
    bvb_d = nc.dram_tensor("bvb", [128, D], F32, kind="ExternalInput")
    mask_d = nc.dram_tensor("mask", [128, 256], F32, kind="ExternalInput")
    y_d = nc.dram_tensor("y", [tq, D], F32, kind="ExternalOutput")

    with tile.TileContext(nc) as tc:
        with (
            tc.tile_pool(name="persist", bufs=1) as pp,
            tc.tile_pool(name="vpool", bufs=1) as vp,
            tc.tile_pool(name="pj_ps", bufs=2, space="PSUM") as pj_ps,
            tc.tile_pool(name="st_ps", bufs=3, space="PSUM") as st_ps,
            tc.tile_pool(name="o_ps", bufs=2, space="PSUM") as o_ps_pool,
            tc.tile_pool(name="ptp", bufs=4) as ptp,
            tc.tile_pool(name="outp", bufs=3) as outp,
            tc.tile_pool(name="finp", bufs=3) as finp,
        ):
            # ---- persistent SBUF inputs
            xT = [pp.tile([128, t], BF16, name=f"xT{k}") for k in range(2)]
            xqT = [pp.tile([128, tq], BF16, name=f"xqT{k}") for k in range(2)]
            wq = [pp.tile([128, D], BF16, name=f"wq{k}") for k in range(2)]
            wk = [pp.tile([128, D], BF16, name=f"wk{k}") for k in range(2)]
            wv = [pp.tile([128, D], BF16, name=f"wv{k}") for k in range(2)]
            bq = [pp.tile([128, 1], F32, name=f"bq{k}") for k in range(2)]
            bk = [pp.tile([128, 1], F32, name=f"bk{k}") for k in range(2)]
            bvb = pp.tile([128, D], F32, name="bvb")
            mask = pp.tile([128, 256], F32, name="mask")
            for k in range(2):
                nc.sync.dma_start(xT[k][:], xT_d[k])
                nc.sync.dma_start(xqT[k][:], xqT_d[k])
                nc.sync.dma_start(wq[k][:], wq_d[k])
                nc.sync.dma_start(wk[k][:], wk_d[k])
                nc.sync.dma_start(wv[k][:], wv_d[k])
                nc.sync.dma_start(bq[k][:], bq_d[k])
                nc.sync.dma_start(bk[k][:], bk_d[k])
            nc.sync.dma_start(bvb[:], bvb_d[:])
            nc.sync.dma_start(mask[:], mask_d[:])

            # ---- projections: KT/QT in [dout, t] layout (bias via DVE)
            KT = [pp.tile([128, t], BF16, name=f"KT{m}") for m in range(2)]
            QT = [pp.tile([128, tq], BF16, name=f"QT{m}") for m in range(2)]
            NBK = min(512, t)
            NBQ = min(512, tq)
            for m in range(2):
                ms = slice(m * 128, (m + 1) * 128)
                for nb in range(t // NBK):
                    ps = pj_ps.tile([128, NBK], F32, name="pj", tag="pj")
                    for k in range(2):
                        nc.tensor.matmul(
                            ps[:],
                            wk[k][:, ms],
                            xT[k][:, nb * NBK : (nb + 1) * NBK],
                            start=(k == 0),
                            stop=(k == 1),
                        )
                    nc.vector.tensor_scalar_add(
                        KT[m][:, nb * NBK : (nb + 1) * NBK], ps[:], bk[m][:]
                    )
                for nb in range(tq // NBQ):
                    ps = pj_ps.tile([128, NBQ], F32, name="pj", tag="pj")
                    for k in range(2):
                        nc.tensor.matmul(
                            ps[:],
                            wq[k][:, ms],
                            xqT[k][:, nb * NBQ : (nb + 1) * NBQ],
                            start=(k == 0),
                            stop=(k == 1),
                        )
                    nc.vector.tensor_scalar_add(
                        QT[m][:, nb * NBQ : (nb + 1) * NBQ], ps[:], bq[m][:]
                    )

            # ---- V projection: natural [s, d] layout + ones column
            V = [vp.tile([128, D + 1], BF16, name=f"v{s}") for s in range(ns)]
            for s in range(ns):
                ps = pj_ps.tile([128, D], F32, name="pj", tag="pj")
                for k in range(2):
                    nc.tensor.matmul(
                        ps[:],
                        xT[k][:, s * 128 : (s + 1) * 128],
                        wv[k][:],
                        start=(k == 0),
                        stop=(k == 1),
                    )
                nc.vector.tensor_add(V[s][:, 0:D], ps[:], bvb[:])
                nc.vector.memset(V[s][:, D : D + 1], 1.0)

            # ---- attention
            exp_t = mybir.ActivationFunctionType.Exp
            for i in range(nq):
                e = 2 * i + 2  # s-tiles this query tile touches
                o_ps = o_ps_pool.tile([128, D + 1], F32, name="ops", tag="ops")
                qs = slice(i * 128, (i + 1) * 128)
                for s in range(e):
                    stp = st_ps.tile([128, 128], F32, name="stp", tag="stp")
                    for k in range(2):
                        nc.tensor.matmul(
                            stp[:],
                            KT[k][:, s * 128 : (s + 1) * 128],
                            QT[k][:, qs],
                            start=(k == 0),
                            stop=(k == 1),
                        )
                    if s == e - 2:
                        nc.vector.tensor_add(stp[:], stp[:], mask[:, 0:128])
                    elif s == e - 1:
                        nc.vector.tensor_add(stp[:], stp[:], mask[:, 128:256])
                    pt = ptp.tile([128, 128], BF16, name="pt", tag="pt")
                    nc.scalar.activation(pt[:], stp[:], exp_t, scale=scale)
                    nc.tensor.matmul(
                        o_ps[:], pt[:], V[s][:], start=(s == 0), stop=(s == e - 1)
                    )
                rec = finp.tile([128, 1], F32, name="rec", tag="rec")
                nc.vector.reciprocal(rec[:], o_ps[:, D : D + 1])
                ob = outp.tile([128, D], F32, name="ob", tag="ob")
                nc.vector.tensor_scalar_mul(ob[:], o_ps[:, 0:D], rec[:])
                nc.sync.dma_start(y_d[i * 128 : (i + 1) * 128, :], ob[:])
    return nc


def build_nc_v2(t: int = T, tq: int = TQ) -> bass.Bass:
    """Quad-grouped attention: 4 query tiles (512 q cols) share each S^T
    matmul / exp pass. Core h owns global q-tiles {8g + 2u + h}; group g
    runs a uniform s-extent of 8g+8 tiles on every core."""
    nq = tq // 128
    ns = t // 128
    ng = nq // 4
    assert t == 2 * tq and nq % 4 == 0
    scale = 1.0 / float(np.sqrt(np.float32(D)))

    nc = bacc.Bacc()
    xT_d = nc.dram_tensor("xT", [2, 128, t], BF16, kind="ExternalInput")
    xqT_d = nc.dram_tensor("xqT", [2, 128, tq], BF16, kind="ExternalInput")
    wq_d = nc.dram_tensor("wq", [2, 128, D], BF16, kind="ExternalInput")
    wk_d = nc.dram_tensor("wk", [2, 128, D], BF16, kind="ExternalInput")
    wv_d = nc.dram_tensor("wv", [2, 128, D], BF16, kind="ExternalInput")
    bq_d = nc.dram_tensor("bq", [2, 128, 1], F32, kind="ExternalInput")
    bk_d = nc.dram_tensor("bk", [2, 128, 1], F32, kind="ExternalInput")
    bvb_d = nc.dram_tensor("bvb", [128, D], F32, kind="ExternalInput")
    mask_d = nc.dram_tensor("mask", [8, 128, 512], F32, kind="ExternalInput")
    y_d = nc.dram_tensor("y", [tq, D], F32, kind="ExternalOutput")

    with tile.TileContext(nc) as tc:
        with (
            tc.tile_pool(name="persist", bufs=1) as pp,
            tc.tile_pool(name="vpool", bufs=1) as vp,
            tc.tile_pool(name="st_ps", bufs=2, space="PSUM") as st_ps,
            tc.tile_pool(name="o_ps", bufs=1, space="PSUM") as o_ps_pool,
            tc.tile_pool(name="ptp", bufs=3) as ptp,
            tc.tile_pool(name="outp", bufs=3) as outp,
            tc.tile_pool(name="finp", bufs=3) as finp,
        ):
            # ---- persistent SBUF inputs
            xT = [pp.tile([128, t], BF16, name=f"xT{k}") for k in range(2)]
            xqT = [pp.tile([128, tq], BF16, name=f"xqT{k}") for k in range(2)]
            wq = [pp.tile([128, D], BF16, name=f"wq{k}") for k in range(2)]
            wk = [pp.tile([128, D], BF16, name=f"wk{k}") for k in range(2)]
            wv = [pp.tile([128, D], BF16, name=f"wv{k}") for k in range(2)]
            bq = [pp.tile([128, 1], F32, name=f"bq{k}") for k in range(2)]
            bk = [pp.tile([128, 1], F32, name=f"bk{k}") for k in range(2)]
            bvb = pp.tile([128, D], F32, name="bvb")
            mask = [pp.tile([128, 512], F32, name=f"mask{r}") for r in range(8)]
            for k in range(2):
                nc.sync.dma_start(xT[k][:], xT_d[k])
                nc.sync.dma_start(xqT[k][:], xqT_d[k])
                nc.sync.dma_start(wq[k][:], wq_d[k])
                nc.sync.dma_start(wk[k][:], wk_d[k])
                nc.sync.dma_start(wv[k][:], wv_d[k])
                nc.sync.dma_start(bq[k][:], bq_d[k])
                nc.sync.dma_start(bk[k][:], bk_d[k])
            nc.sync.dma_start(bvb[:], bvb_d[:])
            for r in range(8):
                nc.sync.dma_start(mask[r][:], mask_d[r])

            KT = [pp.tile([128, t], BF16, name=f"KT{m}") for m in range(2)]
            QT = [pp.tile([128, tq], BF16, name=f"QT{m}") for m in range(2)]
            V = [vp.tile([128, D + 1], BF16, name=f"v{s}") for s in range(ns)]

            # ---- projections in their own PSUM pool (freed before attention)
            with tc.tile_pool(name="pj_ps", bufs=2, space="PSUM") as pj_ps:
                NBK = min(512, t)
                NBQ = min(512, tq)
                for m in range(2):
                    ms = slice(m * 128, (m + 1) * 128)
                    for nb in range(t // NBK):
                        ps = pj_ps.tile([128, NBK], F32, name="pj", tag="pj")
                        for k in range(2):
                            nc.tensor.matmul(
                                ps[:],
                                wk[k][:, ms],
                                xT[k][:, nb * NBK : (nb + 1) * NBK],
                                start=(k == 0),
                                stop=(k == 1),
                            )
                        nc.vector.tensor_scalar_add(
                            KT[m][:, nb * NBK : (nb + 1) * NBK], ps[:], bk[m][:]
                        )
                    for nb in range(tq // NBQ):
                        ps = pj_ps.tile([128, NBQ], F32, name="pj", tag="pj")
                        for k in range(2):
                            nc.tensor.matmul(
                                ps[:],
                                wq[k][:, ms],
                                xqT[k][:, nb * NBQ : (nb + 1) * NBQ],
                                start=(k == 0),
                                stop=(k == 1),
                            )
                        nc.vector.tensor_scalar_add(
                            QT[m][:, nb * NBQ : (nb + 1) * NBQ], ps[:], bq[m][:]
                        )
                for s in range(ns):
                    ps = pj_ps.tile([128, D], F32, name="pj", tag="pj")
                    for k in range(2):
                        nc.tensor.matmul(
                            ps[:],
                            xT[k][:, s * 128 : (s + 1) * 128],
                            wv[k][:],
                            start=(k == 0),
                            stop=(k == 1),
                        )
                    nc.vector.tensor_add(V[s][:, 0:D], ps[:], bvb[:])
                    nc.vector.memset(V[s][:, D : D + 1], 1.0)

            # ---- attention, 512 q cols per group
            exp_t = mybir.ActivationFunctionType.Exp
            att = ctx_att = tc.tile_pool(name="st_ps", bufs=2, space="PSUM")
            st_ps = att.__enter__()
            o_ctx = tc.tile_pool(name="o_ps", bufs=6, space="PSUM")
            o_ps_pool = o_ctx.__enter__()
            for g in range(ng):
                e = 8 * g + 8
                qs = slice(g * 512, (g + 1) * 512)
                o = [
                    o_ps_pool.tile([128, D + 1], F32, name=f"o{u}", tag=f"o{u}")
                    for u in range(4)
                ]
                for s in range(e):
                    stp = st_ps.tile([128, 512], F32, name="stp", tag="stp")
                    for k in range(2):
                        nc.tensor.matmul(
                            stp[:],
                            KT[k][:, s * 128 : (s + 1) * 128],
                            QT[k][:, qs],
                            start=(k == 0),
                            stop=(k == 1),
                        )
                    if s >= 8 * g:
                        nc.vector.tensor_add(stp[:], stp[:], mask[s - 8 * g][:])
                    pt = ptp.tile([128, 512], BF16, name="pt", tag="pt")
                    nc.scalar.activation(pt[:], stp[:], exp_t, scale=scale)
                    for u in range(4):
                        nc.tensor.matmul(
                            o[u][:],
                            pt[:, u * 128 : (u + 1) * 128],
                            V[s][:],
                            start=(s == 0),
                            stop=(s == e - 1),
                        )
                for u in range(4):
                    rec = finp.tile([128, 1], F32, name="rec", tag="rec")
                    nc.vector.reciprocal(rec[:], o[u][:, D : D + 1])
                    ob = outp.tile([128, D], F32, name="ob", tag="ob")
                    nc.vector.tensor_scalar_mul(ob[:], o[u][:, 0:D], rec[:])
                    lrow = (g * 4 + u) * 128
                    nc.sync.dma_start(y_d[lrow : lrow + 128, :], ob[:])
    return nc


def build_nc_v3(t: int = T, tq: int = TQ, st_bufs: int = 4, o_bufs: int = 4, pt_bufs: int = 6, pj_bufs: int = 4) -> bass.Bass:
    """v2 + cheaper masking, less dead work, and walrus-friendly syncs:
    - all constants (weights, biases, masks) packed into two DRAM tensors
      loaded with one DMA each; tiny DVE "absorber" copies pull the DMA
      completion into DVE's vector clock so the bias TensorScalarPtr ops
      carry a single sem wait (walrus rejects multi-wait TS instrs);
    - causal mask applied AFTER exp as a multiplicative 0/1 bf16 mask on
      one 128-col block per diagonal s-tile (DVE bf16 SBUF fast mode);
    - PV matmuls skipped for (s_rel, u) tiles dead on BOTH cores
      (u < floor(s_rel/2)) — the skip pattern is SPMD-uniform;
    - input x DMAs chunked so projections overlap the loads;
    - single-tag o-pool (bufs=6) so group g+1 does not wait on group g's
      finalize."""
    nq = tq // 128
    ns = t // 128
    ng = nq // 4
    assert t == 2 * tq and nq % 4 == 0
    scale = 1.0 / float(np.sqrt(np.float32(D)))

    nc = bacc.Bacc()
    xT_d = nc.dram_tensor("xT", [2, 128, t], BF16, kind="ExternalInput")
    xqT_d = nc.dram_tensor("xqT", [2, 128, tq], BF16, kind="ExternalInput")
    cw_d = nc.dram_tensor("cw", [128, 1536], BF16, kind="ExternalInput")
    cm_d = nc.dram_tensor("cm", [128, 1024], BF16, kind="ExternalInput")
    cf_d = nc.dram_tensor("cf", [128, 260], F32, kind="ExternalInput")
    y_d = nc.dram_tensor("y", [tq, D], F32, kind="ExternalOutput")

    with tile.TileContext(nc) as tc:
        with (
            tc.tile_pool(name="persist", bufs=1) as pp,
            tc.tile_pool(name="vpool", bufs=1) as vp,
            tc.tile_pool(name="ptp", bufs=pt_bufs) as ptp,
            tc.tile_pool(name="outp", bufs=3) as outp,
            tc.tile_pool(name="finp", bufs=4) as finp,
        ):
            # ---- inputs. One sync (HWDGE) queue so transfers complete in
            # priority order: weights -> first x chunks (gates the first
            # projection matmuls) -> rest -> masks (needed ~20us in).
            cw = pp.tile([128, 1536], BF16, name="cw")
            cm = pp.tile([128, 1024], BF16, name="cm")
            cf = pp.tile([128, 260], F32, name="cf")
            xT = [pp.tile([128, t], BF16, name=f"xT{k}") for k in range(2)]
            xqT = [pp.tile([128, tq], BF16, name=f"xqT{k}") for k in range(2)]
            CH = max(512, t // 2)
            nc.sync.dma_start(cw[:], cw_d[:])
            for k in range(2):
                nc.sync.dma_start(xT[k][:, 0:CH], xT_d[k][:, 0:CH])
            nc.sync.dma_start(cf[:], cf_d[:])
            for c0 in range(CH, t, CH):
                for k in range(2):
                    nc.sync.dma_start(xT[k][:, c0 : c0 + CH], xT_d[k][:, c0 : c0 + CH])
            for k in range(2):
                nc.sync.dma_start(xqT[k][:], xqT_d[k])
            nc.sync.dma_start(cm[:], cm_d[:])
            # absorber copies: pull each const DMA's completion into DVE's
            # vector clock so downstream DVE ops carry a single sem wait
            scrb = finp.tile([128, 1], BF16, name="scrb", tag="scrb")
            nc.vector.tensor_copy(scrb[:], cw[:, 0:1])
            scrf = finp.tile([128, 1], F32, name="scrf", tag="scrf")
            nc.vector.tensor_copy(scrf[:], cf[:, 0:1])
            scrm = finp.tile([128, 1], BF16, name="scrm", tag="scrm")
            nc.vector.tensor_copy(scrm[:], cm[:, 0:1])
            wq = [cw[:, 0 + k * 256 : 256 + k * 256] for k in range(2)]
            wk = [cw[:, 512 + k * 256 : 768 + k * 256] for k in range(2)]
            wv = [cw[:, 1024 + k * 256 : 1280 + k * 256] for k in range(2)]
            maskb = [cm[:, r * 128 : (r + 1) * 128] for r in range(8)]
            bq = [cf[:, k : k + 1] for k in range(2)]
            bk = [cf[:, 2 + k : 3 + k] for k in range(2)]
            bvb = cf[:, 4:260]

            # HAM warm-up: garbage matmuls while input DMAs land, so the
            # PE clock gate is already at 8/8 when real work arrives.
            wa = pp.tile([128, 128], BF16, name="wa")
            wb = pp.tile([128, 512], BF16, name="wb")
            nc.vector.memset(wa[:], 0.0)
            nc.vector.memset(wb[:], 0.0)
            with tc.tile_pool(name="warm_ps", bufs=1, space="PSUM") as wps:
                wp_t = wps.tile([128, 512], F32, name="warm")
                for _ in range(20):
                    nc.tensor.matmul(wp_t[:], wa[:], wb[:], start=True, stop=True)

            KT = [pp.tile([128, t], BF16, name=f"KT{m}") for m in range(2)]
            QT = [pp.tile([128, tq], BF16, name=f"QT{m}") for m in range(2)]
            V = [vp.tile([128, D + 1], BF16, name=f"v{s}") for s in range(ns)]

            # ---- projections in their own PSUM pool (freed before attention)
            with tc.tile_pool(name="pj_ps", bufs=pj_bufs, space="PSUM") as pj_ps:
                NBK = min(512, t)
                NBQ = min(512, tq)
                for nb in range(t // NBK):
                    for m in range(2):
                        ms = slice(m * 128, (m + 1) * 128)
                        ps = pj_ps.tile([128, NBK], F32, name="pj", tag="pj")
                        for k in range(2):
                            nc.tensor.matmul(
                                ps[:],
                                wk[k][:, ms],
                                xT[k][:, nb * NBK : (nb + 1) * NBK],
                                start=(k == 0),
                                stop=(k == 1),
                            )
                        nc.vector.tensor_scalar_add(
                            KT[m][:, nb * NBK : (nb + 1) * NBK], ps[:], bk[m]
                        )
                for m in range(2):
                    ms = slice(m * 128, (m + 1) * 128)
                    for nb in range(tq // NBQ):
                        ps = pj_ps.tile([128, NBQ], F32, name="pj", tag="pj")
                        for k in range(2):
                            nc.tensor.matmul(
                                ps[:],
                                wq[k][:, ms],
                                xqT[k][:, nb * NBQ : (nb + 1) * NBQ],
                                start=(k == 0),
                                stop=(k == 1),
                            )
                        nc.vector.tensor_scalar_add(
                            QT[m][:, nb * NBQ : (nb + 1) * NBQ], ps[:], bq[m]
                        )

            # ---- attention, 512 q cols per group
            exp_t = mybir.ActivationFunctionType.Exp
            att = ctx_att = tc.tile_pool(name="st_ps", bufs=st_bufs, space="PSUM")
            st_ps = att.__enter__()
            o_ctx = tc.tile_pool(name="o_ps", bufs=o_bufs, space="PSUM")
            o_ps_pool = o_ctx.__enter__()
            for g in range(ng):
                e = 8 * g + 8
                for s in range(8 * g, min(8 * g + 8, ns)):
                    ps = st_ps.tile([128, D], F32, name="vpj", tag="stp")
                    for k in range(2):
                        nc.tensor.matmul(
                            ps[:],
                            xT[k][:, s * 128 : (s + 1) * 128],
                            wv[k],
                            start=(k == 0),
                            stop=(k == 1),
                        )
                    # bias-free V: since sum_s P = den, (O + den*bv)/den =
                    # O/den + bv, so bv moves to the finalize and this
                    # PSUM->SBUF cast-copy runs on the idle ACT engine
                    nc.scalar.copy(V[s][:, 0:D], ps[:])
                    nc.vector.memset(V[s][:, D : D + 1], 1.0)
                qs = slice(g * 512, (g + 1) * 512)
                o = [
                    o_ps_pool.tile([128, D + 1], F32, name=f"o{u}", tag="o")
                    for u in range(4)
                ]
                for s in range(e):
                    s_rel = s - 8 * g
                    u0 = max(s_rel, 0) // 2  # first live 128-col block
                    c0 = u0 * 128
                    stp = st_ps.tile([128, 512], F32, name="stp", tag="stp")
                    for k in range(2):
                        nc.tensor.matmul(
                            stp[:, c0:512],
                            KT[k][:, s * 128 : (s + 1) * 128],
                            QT[k][:, g * 512 + c0 : (g + 1) * 512],
                            start=(k == 0),
                            stop=(k == 1),
                        )
                    pt = ptp.tile([128, 512], BF16, name="pt", tag="pt")
                    nc.scalar.activation(
                        pt[:, c0:512], stp[:, c0:512], exp_t, scale=scale
                    )
                    if s_rel >= 0:
                        nc.vector.tensor_mul(
                            pt[:, c0 : c0 + 128],
                            pt[:, c0 : c0 + 128],
                            maskb[s_rel],
                        )
                    for u in range(4):
                        if s_rel >= 0 and u < s_rel // 2:
                            continue  # dead on every core
                        nc.tensor.matmul(
                            o[u][:],
                            pt[:, u * 128 : (u + 1) * 128],
                            V[s][:],
                            start=(s == 0),
                            stop=(s == 8 * g + 2 * u + 1),
                        )
                for u in range(4):
                    rec = finp.tile([128, 1], F32, name="rec", tag="rec")
                    nc.vector.reciprocal(rec[:], o[u][:, D : D + 1])
                    ob = outp.tile([128, D], F32, name="ob", tag="ob")
                    nc.vector.scalar_tensor_tensor(
                        ob[:],
                        o[u][:, 0:D],
                        rec[:],
                        bvb,
                        mybir.AluOpType.mult,
                        mybir.AluOpType.add,
                    )
                    lrow = (g * 4 + u) * 128
                    nc.sync.dma_start(y_d[lrow : lrow + 128, :], ob[:])
            o_ctx.__exit__(None, None, None)
            ctx_att.__exit__(None, None, None)
    return nc


def build_nc_v4(
    t: int = T,
    tq: int = TQ,
    st_bufs: int = 3,
    o_bufs: int = 5,
    pt_bufs: int = 6,
    warm_n: int = 12,
) -> bass.Bass:
    """v3 + fp8 S^T and a fully interleaved projection/attention pipeline.

    - S^T = K^T·Q runs as ONE fp8e4 DoubleRow matmul per (s, group): the PE
      contracts all 256 d-rows in a single pass (2 rows/cycle), halving the
      S cost vs the bf16 2-pass version. Q/K are projected in bf16 precision
      (PSUM f32) and only quantized at the PSUM->SBUF cast (measured rel err
      ~1.1e-2 vs fp32 reference, tolerance 2e-2). P and V stay bf16 (fp8
      there costs ~3.6% rms -> fails tolerance).
    - K's bias is dropped entirely: softmax is invariant to the row-constant
      q·bk term, so only bq (via the Q cast) matters.
    - Projections are interleaved per query-group g and allocate their PSUM
      from the SAME pool/tag as the o accumulators (every slot is a full
      bank anyway): the 4-deep rotation hides the ~0.7us cast latency that
      a dedicated 2-buf pool exposed, and st_ps gets 4 banks for deeper S
      lookahead. Casts/copies alternate DVE/ACT to split the backlog.
    - Inputs stream on BOTH hardware DGE queues (SP + ACT); transfers on one
      queue serialize (each push waits the previous transfer's completion
      semaphore), so the host packs x/xq/consts into few, need-ordered
      transfers: [cwm | xq-g0 | cf | xq-rest] on ACT, 4 x group-chunks on SP.
    - o[u] finalize (reciprocal + scale + bias + output DMA) is emitted
      inside the s-loop right after u's stopping matmul, so the tail after
      the last PV matmul is just one finalize chain.
    """
    nq = tq // 128
    ns = t // 128
    ng = nq // 4
    assert t == 2 * tq and nq % 4 == 0
    scale = 1.0 / float(np.sqrt(np.float32(D)))
    FP8 = mybir.dt.float8e4

    nc = bacc.Bacc()
    xT_d = nc.dram_tensor("xT", [128, 2, t], BF16, kind="ExternalInput")
    xqT_d = nc.dram_tensor("xqT", [128, 2, tq], BF16, kind="ExternalInput")
    cwm_d = nc.dram_tensor("cwm", [128, 2560], BF16, kind="ExternalInput")
    cf_d = nc.dram_tensor("cf", [128, 258], F32, kind="ExternalInput")
    y_d = nc.dram_tensor("y", [tq, D], F32, kind="ExternalOutput")

    with tile.TileContext(nc) as tc:
        with (
            tc.tile_pool(name="persist", bufs=1) as pp,
            tc.tile_pool(name="vpool", bufs=1) as vp,
            tc.tile_pool(name="ptp", bufs=pt_bufs) as ptp,
            tc.tile_pool(name="outp", bufs=3) as outp,
            tc.tile_pool(name="finp", bufs=4) as finp,
        ):
            cwm = pp.tile([128, 2560], BF16, name="cwm")
            cf = pp.tile([128, 258], F32, name="cf")
            xTa = pp.tile([128, 2 * t], BF16, name="xTa")
            xqTa = pp.tile([128, 2 * tq], BF16, name="xqTa")
            xTv = xTa[:].rearrange("p (k c) -> p k c", k=2)
            xqTv = xqTa[:].rearrange("p (k c) -> p k c", k=2)
            xT = [xTa[:, k * t : (k + 1) * t] for k in range(2)]
            xqT = [xqTa[:, k * tq : (k + 1) * tq] for k in range(2)]

            def xts(k, a, b):
                return xT[k][:, a:b]

            def xqs(k, a, b):
                return xqT[k][:, a:b]
            KT8 = pp.tile([128, 2 * t], FP8, name="KT8")
            QT8 = pp.tile([128, 2 * tq], FP8, name="QT8")
            KT8v = KT8[:].rearrange("p (m c) -> p m c", m=2)
            QT8v = QT8[:].rearrange("p (m c) -> p m c", m=2)
            V = [vp.tile([128, D + 1], BF16, name=f"v{s}") for s in range(ns)]

            # ---- input DMA: the two DGE queues share ~358GB/s of HBM and
            # the ACT-side queue starts late and runs at ~half rate, so the
            # group-0-critical set streams need-ordered on the fast SP
            # queue; only the latest-needed bulk rides the ACT queue.
            nc.scalar.dma_start(xqTv[:, :, 0:512], xqT_d[:, :, 0:512])
            nc.scalar.dma_start(cf[:], cf_d[:])
            nc.sync.dma_start(cwm[:], cwm_d[:])
            nc.sync.dma_start(xTv[:, :, 0:1024], xT_d[:, :, 0:1024])
            nc.sync.dma_start(xTv[:, :, 1024:2048], xT_d[:, :, 1024:2048])
            nc.sync.dma_start(xqTv[:, :, 512:1024], xqT_d[:, :, 512:1024])
            nc.sync.dma_start(xTv[:, :, 2048:3072], xT_d[:, :, 2048:3072])
            nc.sync.dma_start(xqTv[:, :, 1024:tq], xqT_d[:, :, 1024:tq])
            nc.sync.dma_start(xTv[:, :, 3072:t], xT_d[:, :, 3072:t])

            wq = [cwm[:, 0 + k * 256 : 256 + k * 256] for k in range(2)]
            wk = [cwm[:, 512 + k * 256 : 768 + k * 256] for k in range(2)]
            wv = [cwm[:, 1024 + k * 256 : 1280 + k * 256] for k in range(2)]
            maskb = [cwm[:, 1536 + r * 128 : 1536 + (r + 1) * 128] for r in range(8)]
            bq = [cf[:, k : k + 1] for k in range(2)]
            bvb = cf[:, 2:258]

            # warm-up garbage matmuls bridge the input-DMA wait so the PE
            # HAM clock gate is at 8/8 when real work arrives
            wa = pp.tile([128, 128], BF16, name="wa")
            wb = pp.tile([128, 512], BF16, name="wb")
            nc.vector.memset(wa[:], 0.0)
            nc.vector.memset(wb[:], 0.0)
            with tc.tile_pool(name="warm_ps", bufs=1, space="PSUM") as wps:
                wp_t = wps.tile([128, 512], F32, name="warm")
                for _ in range(warm_n):
                    nc.tensor.matmul(wp_t[:], wa[:], wb[:], start=True, stop=True)

            # denominator ones-column, set once per V tile (off critical path)
            for s in range(ns):
                nc.vector.memset(V[s][:, D : D + 1], 1.0)
            # absorber copies pull const DMA completions into each consumer
            # engine's vector clock (single extra sem wait per consumer op)
            scrb = finp.tile([128, 1], BF16, name="scrb", tag="scrb")
            nc.vector.tensor_copy(scrb[:], cwm[:, 0:1])
            scrf = finp.tile([128, 1], F32, name="scrf", tag="scrf")
            nc.vector.tensor_copy(scrf[:], cf[:, 0:1])
            scrg = finp.tile([128, 1], F32, name="scrg", tag="scrg")
            nc.scalar.copy(scrg[:], cf[:, 0:1])

            exp_t = mybir.ActivationFunctionType.Exp
            id_t = mybir.ActivationFunctionType.Identity
            with (
                tc.tile_pool(name="st_ps", bufs=st_bufs, space="PSUM") as st_ps,
                tc.tile_pool(name="o_ps", bufs=o_bufs, space="PSUM") as o_ps_pool,
            ):
                def q_proj(g):
                    for m in range(2):
                        ms = slice(m * 128, (m + 1) * 128)
                        ps = o_ps_pool.tile([128, 512], F32, name="pj", tag="o")
                        for k in range(2):
                            nc.tensor.matmul(
                                ps[:], wq[k][:, ms],
                                xqs(k, g * 512, (g + 1) * 512),
                                start=(k == 0), stop=(k == 1),
                            )
                        dst = QT8[:, m * tq + g * 512 : m * tq + (g + 1) * 512]
                        if m == 0:
                            nc.vector.tensor_scalar_add(dst, ps[:], bq[m])
                        else:
                            nc.scalar.activation(dst, ps[:], id_t, bias=bq[m])

                def k_proj(g):
                    # no bias: q·bk is row-constant, softmax-invariant
                    for nb in range(2):
                        ca = g * 1024 + nb * 512
                        for m in range(2):
                            ms = slice(m * 128, (m + 1) * 128)
                            ps = o_ps_pool.tile([128, 512], F32, name="pj", tag="o")
                            for k in range(2):
                                nc.tensor.matmul(
                                    ps[:], wk[k][:, ms], xts(k, ca, ca + 512),
                                    start=(k == 0), stop=(k == 1),
                                )
                            dst = KT8[
                                :,
                                m * t + g * 1024 + nb * 512 :
                                m * t + g * 1024 + (nb + 1) * 512,
                            ]
                            if (2 * nb + m) % 2 == 0:
                                nc.vector.tensor_copy(dst, ps[:])
                            else:
                                nc.scalar.copy(dst, ps[:])

                def v_proj(g):
                    # bias-free; bv is added in the finalize
                    for s in range(8 * g, 8 * g + 8):
                        ps = o_ps_pool.tile([128, D], F32, name="vpj", tag="o")
                        for k in range(2):
                            nc.tensor.matmul(
                                ps[:], xts(k, s * 128, (s + 1) * 128), wv[k],
                                start=(k == 0), stop=(k == 1),
                            )
                        if s % 2 == 0:
                            nc.scalar.copy(V[s][:, 0:D], ps[:])
                        else:
                            nc.vector.tensor_copy(V[s][:, 0:D], ps[:])

                for g in range(ng):
                    e = 8 * g + 8
                    q_proj(g), k_proj(g), v_proj(g)
                    if g == 0:
                        scrm = finp.tile([128, 1], BF16, name="scrm", tag="scrm")
                        nc.vector.tensor_copy(scrm[:], cwm[:, 1536:1537])
                    # ---- attention for group g
                    o = [
                        o_ps_pool.tile([128, D + 1], F32, name=f"o{u}", tag="o")
                        for u in range(4)
                    ]
                    for s in range(e):
                        s_rel = s - 8 * g
                        u0 = max(s_rel, 0) // 2
                        c0 = u0 * 128
                        stp = st_ps.tile([128, 512], F32, name="stp", tag="stp")
                        nc.tensor.matmul(
                            stp[:, c0:512],
                            KT8v[:, :, s * 128 : (s + 1) * 128],
                            QT8v[:, :, g * 512 + c0 : (g + 1) * 512],
                            start=True, stop=True,
                            perf_mode=mybir.MatmulPerfMode.DoubleRow,
                        )
                        pt = ptp.tile([128, 512], BF16, name="pt", tag="pt")
                        nc.scalar.activation(
                            pt[:, c0:512], stp[:, c0:512], exp_t, scale=scale
                        )
                        if s_rel >= 0:
                            nc.vector.tensor_mul(
                                pt[:, c0 : c0 + 128],
                                pt[:, c0 : c0 + 128],
                                maskb[s_rel],
                            )
                        # u0's P block waits on the DVE mask -> run it last
                        for u in list(range(u0 + 1, 4)) + [u0]:
                            nc.tensor.matmul(
                                o[u][:],
                                pt[:, u * 128 : (u + 1) * 128],
                                V[s][:],
                                start=(s == 0),
                                stop=(s == 8 * g + 2 * u + 1),
                            )
                            if s == 8 * g + 2 * u + 1:
                                rec = finp.tile([128, 1], F32, name="rec", tag="rec")
                                nc.vector.reciprocal(rec[:], o[u][:, D : D + 1])
                                ob = outp.tile([128, D], F32, name="ob", tag="ob")
                                nc.vector.scalar_tensor_tensor(
                                    ob[:], o[u][:, 0:D], rec[:], bvb,
                                    mybir.AluOpType.mult, mybir.AluOpType.add,
                                )
                                lrow = (g * 4 + u) * 128
                                nc.sync.dma_start(y_d[lrow : lrow + 128, :], ob[:])
    return nc


def build_nc_v5(
    t: int = T,
    tq: int = TQ,
    st_bufs: int = 3,
    o_bufs: int = 5,
    pt_bufs: int = 6,
    warm_n: int = 12,
) -> bass.Bass:
    """v4 + engine rebalance: ACT runs ONLY the 80 exp activations (its
    ~0.83ns/col + ~143ns/op makes it the co-bottleneck in v4 where it also
    carried K/V/Q-bias casts). All projection PSUM->SBUF casts move to DVE
    and Pool (gpsimd):
      - K fp8 casts + Q bias-adds -> DVE
      - V bf16 casts + ones-col memsets -> Pool
    Everything else identical to v4.
    """
    nq = tq // 128
    ns = t // 128
    ng = nq // 4
    assert t == 2 * tq and nq % 4 == 0
    scale = 1.0 / float(np.sqrt(np.float32(D)))
    FP8 = mybir.dt.float8e4

    nc = bacc.Bacc()
    xT_d = nc.dram_tensor("xT", [128, 2, t], BF16, kind="ExternalInput")
    xqT_d = nc.dram_tensor("xqT", [128, 2, tq], BF16, kind="ExternalInput")
    cwm_d = nc.dram_tensor("cwm", [128, 2560], BF16, kind="ExternalInput")
    cf_d = nc.dram_tensor("cf", [128, 258], F32, kind="ExternalInput")
    y_d = nc.dram_tensor("y", [tq, D], F32, kind="ExternalOutput")

    with tile.TileContext(nc) as tc:
        with (
            tc.tile_pool(name="persist", bufs=1) as pp,
            tc.tile_pool(name="vpool", bufs=1) as vp,
            tc.tile_pool(name="ptp", bufs=pt_bufs) as ptp,
            tc.tile_pool(name="outp", bufs=3) as outp,
            tc.tile_pool(name="finp", bufs=4) as finp,
        ):
            cwm = pp.tile([128, 2560], BF16, name="cwm")
            cf = pp.tile([128, 258], F32, name="cf")
            xTa = pp.tile([128, 2 * t], BF16, name="xTa")
            xqTa = pp.tile([128, 2 * tq], BF16, name="xqTa")
            xTv = xTa[:].rearrange("p (k c) -> p k c", k=2)
            xqTv = xqTa[:].rearrange("p (k c) -> p k c", k=2)
            xT = [xTa[:, k * t : (k + 1) * t] for k in range(2)]
            xqT = [xqTa[:, k * tq : (k + 1) * tq] for k in range(2)]

            def xts(k, a, b):
                return xT[k][:, a:b]

            def xqs(k, a, b):
                return xqT[k][:, a:b]
            KT8 = pp.tile([128, 2 * t], FP8, name="KT8")
            QT8 = pp.tile([128, 2 * tq], FP8, name="QT8")
            KT8v = KT8[:].rearrange("p (m c) -> p m c", m=2)
            QT8v = QT8[:].rearrange("p (m c) -> p m c", m=2)
            V = [vp.tile([128, D + 1], BF16, name=f"v{s}") for s in range(ns)]

            nc.scalar.dma_start(xqTv[:, :, 0:512], xqT_d[:, :, 0:512])
            nc.scalar.dma_start(cf[:], cf_d[:])
            nc.sync.dma_start(cwm[:], cwm_d[:])
            nc.sync.dma_start(xTv[:, :, 0:1024], xT_d[:, :, 0:1024])
            nc.sync.dma_start(xTv[:, :, 1024:2048], xT_d[:, :, 1024:2048])
            nc.sync.dma_start(xqTv[:, :, 512:1024], xqT_d[:, :, 512:1024])
            nc.sync.dma_start(xTv[:, :, 2048:3072], xT_d[:, :, 2048:3072])
            nc.sync.dma_start(xqTv[:, :, 1024:tq], xqT_d[:, :, 1024:tq])
            nc.sync.dma_start(xTv[:, :, 3072:t], xT_d[:, :, 3072:t])

            wq = [cwm[:, 0 + k * 256 : 256 + k * 256] for k in range(2)]
            wk = [cwm[:, 512 + k * 256 : 768 + k * 256] for k in range(2)]
            wv = [cwm[:, 1024 + k * 256 : 1280 + k * 256] for k in range(2)]
            maskb = [cwm[:, 1536 + r * 128 : 1536 + (r + 1) * 128] for r in range(8)]
            bq = [cf[:, k : k + 1] for k in range(2)]
            bvb = cf[:, 2:258]

            wa = pp.tile([128, 128], BF16, name="wa")
            wb = pp.tile([128, 512], BF16, name="wb")
            nc.vector.memset(wa[:], 0.0)
            nc.vector.memset(wb[:], 0.0)
            with tc.tile_pool(name="warm_ps", bufs=1, space="PSUM") as wps:
                wp_t = wps.tile([128, 512], F32, name="warm")
                for _ in range(warm_n):
                    nc.tensor.matmul(wp_t[:], wa[:], wb[:], start=True, stop=True)

            # ones columns on Pool (keeps DVE/ACT clear)
            for s in range(ns):
                nc.gpsimd.memset(V[s][:, D : D + 1], 1.0)
            # absorber copies pull const DMA completions into each consumer
            # engine's vector clock (single extra sem wait per consumer op)
            scrb = finp.tile([128, 1], BF16, name="scrb", tag="scrb")
            nc.vector.tensor_copy(scrb[:], cwm[:, 0:1])
            scrf = finp.tile([128, 1], F32, name="scrf", tag="scrf")
            nc.vector.tensor_copy(scrf[:], cf[:, 0:1])
            scrg = finp.tile([128, 1], F32, name="scrg", tag="scrg")
            nc.scalar.copy(scrg[:], cf[:, 0:1])
            scrp = finp.tile([128, 1], BF16, name="scrp", tag="scrp")
            nc.gpsimd.tensor_copy(scrp[:], cwm[:, 1024:1025])

            exp_t = mybir.ActivationFunctionType.Exp
            with (
                tc.tile_pool(name="st_ps", bufs=st_bufs, space="PSUM") as st_ps,
                tc.tile_pool(name="o_ps", bufs=o_bufs, space="PSUM") as o_ps_pool,
            ):
                def q_proj(g):
                    for m in range(2):
                        ms = slice(m * 128, (m + 1) * 128)
                        ps = o_ps_pool.tile([128, 512], F32, name="pj", tag="o")
                        for k in range(2):
                            nc.tensor.matmul(
                                ps[:], wq[k][:, ms],
                                xqs(k, g * 512, (g + 1) * 512),
                                start=(k == 0), stop=(k == 1),
                            )
                        dst = QT8[:, m * tq + g * 512 : m * tq + (g + 1) * 512]
                        nc.vector.tensor_scalar_add(dst, ps[:], bq[m])

                def k_proj(g):
                    # no bias: q·bk is row-constant, softmax-invariant
                    for nb in range(2):
                        ca = g * 1024 + nb * 512
                        for m in range(2):
                            ms = slice(m * 128, (m + 1) * 128)
                            ps = o_ps_pool.tile([128, 512], F32, name="pj", tag="o")
                            for k in range(2):
                                nc.tensor.matmul(
                                    ps[:], wk[k][:, ms], xts(k, ca, ca + 512),
                                    start=(k == 0), stop=(k == 1),
                                )
                            dst = KT8[
                                :,
                                m * t + g * 1024 + nb * 512 :
                                m * t + g * 1024 + (nb + 1) * 512,
                            ]
                            nc.vector.tensor_copy(dst, ps[:])

                def v_proj(g):
                    # bias-free; bv is added in the finalize
                    for s in range(8 * g, 8 * g + 8):
                        ps = o_ps_pool.tile([128, D], F32, name="vpj", tag="o")
                        for k in range(2):
                            nc.tensor.matmul(
                                ps[:], xts(k, s * 128, (s + 1) * 128), wv[k],
                                start=(k == 0), stop=(k == 1),
                            )
                        # GPSIMD cannot access PSUM -> DVE
                        nc.vector.tensor_copy(V[s][:, 0:D], ps[:])

                for g in range(ng):
                    e = 8 * g + 8
                    q_proj(g), k_proj(g), v_proj(g)
                    if g == 0:
                        scrm = finp.tile([128, 1], BF16, name="scrm", tag="scrm")
                        nc.vector.tensor_copy(scrm[:], cwm[:, 1536:1537])
                    # ---- attention for group g
                    o = [
                        o_ps_pool.tile([128, D + 1], F32, name=f"o{u}", tag="o")
                        for u in range(4)
                    ]
                    for s in range(e):
                        s_rel = s - 8 * g
                        u0 = max(s_rel, 0) // 2
                        c0 = u0 * 128
                        stp = st_ps.tile([128, 512], F32, name="stp", tag="stp")
                        nc.tensor.matmul(
                            stp[:, c0:512],
                            KT8v[:, :, s * 128 : (s + 1) * 128],
                            QT8v[:, :, g * 512 + c0 : (g + 1) * 512],
                            start=True, stop=True,
                            perf_mode=mybir.MatmulPerfMode.DoubleRow,
                        )
                        pt = ptp.tile([128, 512], BF16, name="pt", tag="pt")
                        nc.scalar.activation(
                            pt[:, c0:512], stp[:, c0:512], exp_t, scale=scale
                        )
                        if s_rel >= 0:
                            nc.vector.tensor_mul(
                                pt[:, c0 : c0 + 128],
                                pt[:, c0 : c0 + 128],
                                maskb[s_rel],
                            )
                        # u0's P block waits on the DVE mask -> run it last
                        for u in list(range(u0 + 1, 4)) + [u0]:
                            nc.tensor.matmul(
                                o[u][:],
                                pt[:, u * 128 : (u + 1) * 128],
                                V[s][:],
                                start=(s == 0),
                                stop=(s == 8 * g + 2 * u + 1),
                            )
                            if s == 8 * g + 2 * u + 1:
                                rec = finp.tile([128, 1], F32, name="rec", tag="rec")
                                nc.vector.reciprocal(rec[:], o[u][:, D : D + 1])
                                ob = outp.tile([128, D], F32, name="ob", tag="ob")
                                nc.vector.scalar_tensor_tensor(
                                    ob[:], o[u][:, 0:D], rec[:], bvb,
                                    mybir.AluOpType.mult, mybir.AluOpType.add,
                                )
                                lrow = (g * 4 + u) * 128
                                nc.sync.dma_start(y_d[lrow : lrow + 128, :], ob[:])
    return nc


def build_nc_v6(
    t: int = T,
    tq: int = TQ,
    st_bufs: int = 3,
    o_bufs: int = 5,
    pt_bufs: int = 6,
    warm_n: int = 12,
) -> bass.Bass:
    """v5 + QK-fold: the K projection is algebraically eliminated.

    S = (xq Wq + bq)(x Wk)^T  [bk dropped: softmax-invariant]
      = xq (Wq Wk^T) x^T + (x Wk bq)^T-broadcast
      = Yq x^T + beta_s
    with M = Wq Wk^T and c = Wk bq folded on the HOST (weight-only /
    thin matvec), beta_s lands in the exp's per-partition bias operand
    (out = exp(in*scale + bias), bias pre-scaled by `scale` host-side).
    The kernel computes Yq = xq M on PE (bf16, same cost as the old Q
    projection), casts to fp8, and runs S^T = DoubleRow(x8T, Yq8) with
    the host-quantized fp8 copy of x as the stationary side. Precision
    is unchanged vs v4/v5 (x8 plays k8's role, Yq8 plays q8's).

    PE col count drops from ~139k to ~123k (K proj gone), DVE loses the
    16 K-cast ops, ACT stays exp-only.
    """
    nq = tq // 128
    ns = t // 128
    ng = nq // 4
    assert t == 2 * tq and nq % 4 == 0
    scale = 1.0 / float(np.sqrt(np.float32(D)))
    FP8 = mybir.dt.float8e4

    nc = bacc.Bacc()
    xT_d = nc.dram_tensor("xT", [128, 2, t], BF16, kind="ExternalInput")
    x8T_d = nc.dram_tensor("x8T", [128, 2, t], FP8, kind="ExternalInput")
    xqT_d = nc.dram_tensor("xqT", [128, 2, tq], BF16, kind="ExternalInput")
    cwm_d = nc.dram_tensor("cwm", [128, 2048], BF16, kind="ExternalInput")
    cf_d = nc.dram_tensor("cf", [128, 288], F32, kind="ExternalInput")
    y_d = nc.dram_tensor("y", [tq, D], F32, kind="ExternalOutput")

    with tile.TileContext(nc) as tc:
        with (
            tc.tile_pool(name="persist", bufs=1) as pp,
            tc.tile_pool(name="vpool", bufs=1) as vp,
            tc.tile_pool(name="ptp", bufs=pt_bufs) as ptp,
            tc.tile_pool(name="outp", bufs=3) as outp,
            tc.tile_pool(name="finp", bufs=4) as finp,
        ):
            cwm = pp.tile([128, 2048], BF16, name="cwm")
            cf = pp.tile([128, 288], F32, name="cf")
            xTa = pp.tile([128, 2 * t], BF16, name="xTa")
            x8Ta = pp.tile([128, 2 * t], FP8, name="x8Ta")
            xqTa = pp.tile([128, 2 * tq], BF16, name="xqTa")
            xTv = xTa[:].rearrange("p (k c) -> p k c", k=2)
            x8Tv = x8Ta[:].rearrange("p (k c) -> p k c", k=2)
            xqTv = xqTa[:].rearrange("p (k c) -> p k c", k=2)
            xT = [xTa[:, k * t : (k + 1) * t] for k in range(2)]
            xqT = [xqTa[:, k * tq : (k + 1) * tq] for k in range(2)]

            def xts(k, a, b):
                return xT[k][:, a:b]

            def xqs(k, a, b):
                return xqT[k][:, a:b]
            Yq8 = pp.tile([128, 2 * tq], FP8, name="Yq8")
            Yq8v = Yq8[:].rearrange("p (m c) -> p m c", m=2)
            V = [vp.tile([128, D + 1], BF16, name=f"v{s}") for s in range(ns)]

            # ---- input DMA, need-ordered. ACT-side queue: group-0 xq +
            # constants; SP queue: everything else in group order.
            nc.scalar.dma_start(xqTv[:, :, 0:512], xqT_d[:, :, 0:512])
            nc.scalar.dma_start(cf[:], cf_d[:])
            nc.sync.dma_start(cwm[:], cwm_d[:])
            nc.sync.dma_start(x8Tv[:, :, 0:1024], x8T_d[:, :, 0:1024])
            nc.sync.dma_start(xTv[:, :, 0:1024], xT_d[:, :, 0:1024])
            nc.sync.dma_start(xTv[:, :, 1024:2048], xT_d[:, :, 1024:2048])
            nc.sync.dma_start(x8Tv[:, :, 1024:2048], x8T_d[:, :, 1024:2048])
            nc.sync.dma_start(xqTv[:, :, 512:1024], xqT_d[:, :, 512:1024])
            nc.sync.dma_start(xTv[:, :, 2048:3072], xT_d[:, :, 2048:3072])
            nc.sync.dma_start(x8Tv[:, :, 2048:3072], x8T_d[:, :, 2048:3072])
            nc.sync.dma_start(xqTv[:, :, 1024:tq], xqT_d[:, :, 1024:tq])
            nc.sync.dma_start(xTv[:, :, 3072:t], xT_d[:, :, 3072:t])
            nc.sync.dma_start(x8Tv[:, :, 3072:t], x8T_d[:, :, 3072:t])

            Mh = [cwm[:, 0 + k * 256 : 256 + k * 256] for k in range(2)]
            wv = [cwm[:, 512 + k * 256 : 768 + k * 256] for k in range(2)]
            maskb = [cwm[:, 1024 + r * 128 : 1024 + (r + 1) * 128] for r in range(8)]
            beta = [cf[:, s : s + 1] for s in range(ns)]
            bvb = cf[:, 32:288]

            # warm-up garbage matmuls bridge the input-DMA wait so the PE
            # p-state/clock gate is hot when real work arrives
            wa = pp.tile([128, 128], BF16, name="wa")
            wb = pp.tile([128, 512], BF16, name="wb")
            nc.vector.memset(wa[:], 0.0)
            nc.vector.memset(wb[:], 0.0)
            with tc.tile_pool(name="warm_ps", bufs=1, space="PSUM") as wps:
                wp_t = wps.tile([128, 512], F32, name="warm")
                for _ in range(warm_n):
                    nc.tensor.matmul(wp_t[:], wa[:], wb[:], start=True, stop=True)

            # ones columns (denominator trick) on Pool, off everyone's path
            for s in range(ns):
                nc.gpsimd.memset(V[s][:, D : D + 1], 1.0)
            # absorber copies pull const DMA completions into each consumer
            # engine's vector clock (single extra sem wait per consumer op)
            scrb = finp.tile([128, 1], BF16, name="scrb", tag="scrb")
            nc.vector.tensor_copy(scrb[:], cwm[:, 0:1])
            scrf = finp.tile([128, 1], F32, name="scrf", tag="scrf")
            nc.vector.tensor_copy(scrf[:], cf[:, 0:1])
            scrg = finp.tile([128, 1], F32, name="scrg", tag="scrg")
            nc.scalar.copy(scrg[:], cf[:, 0:1])

            exp_t = mybir.ActivationFunctionType.Exp
            with (
                tc.tile_pool(name="st_ps", bufs=st_bufs, space="PSUM") as st_ps,
                tc.tile_pool(name="o_ps", bufs=o_bufs, space="PSUM") as o_ps_pool,
            ):
                def yq_proj(g):
                    # Yq = xq M, no bias (bq lives in beta); fp8 cast on DVE
                    for m in range(2):
                        ms = slice(m * 128, (m + 1) * 128)
                        ps = o_ps_pool.tile([128, 512], F32, name="pj", tag="o")
                        for k in range(2):
                            nc.tensor.matmul(
                                ps[:], Mh[k][:, ms],
                                xqs(k, g * 512, (g + 1) * 512),
                                start=(k == 0), stop=(k == 1),
                            )
                        dst = Yq8[:, m * tq + g * 512 : m * tq + (g + 1) * 512]
                        nc.vector.tensor_copy(dst, ps[:])

                def v_proj(g):
                    # bias-free; bv is added in the finalize
                    for s in range(8 * g, 8 * g + 8):
                        ps = o_ps_pool.tile([128, D], F32, name="vpj", tag="o")
                        for k in range(2):
                            nc.tensor.matmul(
                                ps[:], xts(k, s * 128, (s + 1) * 128), wv[k],
                                start=(k == 0), stop=(k == 1),
                            )
                        nc.vector.tensor_copy(V[s][:, 0:D], ps[:])

                for g in range(ng):
                    e = 8 * g + 8
                    yq_proj(g), v_proj(g)
                    if g == 0:
                        scrm = finp.tile([128, 1], BF16, name="scrm", tag="scrm")
                        nc.vector.tensor_copy(scrm[:], cwm[:, 1024:1025])
                    # ---- attention for group g
                    o = [
                        o_ps_pool.tile([128, D + 1], F32, name=f"o{u}", tag="o")
                        for u in range(4)
                    ]
                    for s in range(e):
                        s_rel = s - 8 * g
                        u0 = max(s_rel, 0) // 2
                        c0 = u0 * 128
                        stp = st_ps.tile([128, 512], F32, name="stp", tag="stp")
                        nc.tensor.matmul(
                            stp[:, c0:512],
                            x8Tv[:, :, s * 128 : (s + 1) * 128],
                            Yq8v[:, :, g * 512 + c0 : (g + 1) * 512],
                            start=True, stop=True,
                            perf_mode=mybir.MatmulPerfMode.DoubleRow,
                        )
                        pt = ptp.tile([128, 512], BF16, name="pt", tag="pt")
                        nc.scalar.activation(
                            pt[:, c0:512], stp[:, c0:512], exp_t,
                            bias=beta[s], scale=scale,
                        )
                        if s_rel >= 0:
                            nc.vector.tensor_mul(
                                pt[:, c0 : c0 + 128],
                                pt[:, c0 : c0 + 128],
                                maskb[s_rel],
                            )
                        # u0's P block waits on the DVE mask -> run it last
                        for u in list(range(u0 + 1, 4)) + [u0]:
                            nc.tensor.matmul(
                                o[u][:],
                                pt[:, u * 128 : (u + 1) * 128],
                                V[s][:],
                                start=(s == 0),
                                stop=(s == 8 * g + 2 * u + 1),
                            )
                            if s == 8 * g + 2 * u + 1:
                                rec = finp.tile([128, 1], F32, name="rec", tag="rec")
                                nc.vector.reciprocal(rec[:], o[u][:, D : D + 1])
                                ob = outp.tile([128, D], F32, name="ob", tag="ob")
                                nc.vector.scalar_tensor_tensor(
                                    ob[:], o[u][:, 0:D], rec[:], bvb,
                                    mybir.AluOpType.mult, mybir.AluOpType.add,
                                )
                                lrow = (g * 4 + u) * 128
                                nc.sync.dma_start(y_d[lrow : lrow + 128, :], ob[:])
    return nc


def prep_inputs(
    x, Wq, bq, Wk, bk, Wv, bv, t: int = T, n_cores: int = N_CORES, version: int = 1
):
    """Per-core input maps (host-side shard / transpose / cast)."""
    x = np.asarray(x, dtype=np.float32)
    b_dim = x.shape[0]
    tq = t // 2
    nq = tq // 128
    shared = {}
    for name, w in (("wq", Wq), ("wk", Wk), ("wv", Wv)):
        shared[name] = np.ascontiguousarray(
            np.asarray(w, np.float32).astype(NPBF16).reshape(2, 128, D)
        )
    shared["bq"] = np.ascontiguousarray(
        np.asarray(bq, np.float32).reshape(2, 128, 1)
    )
    shared["bk"] = np.ascontiguousarray(
        np.asarray(bk, np.float32).reshape(2, 128, 1)
    )
    shared["bvb"] = np.ascontiguousarray(
        np.broadcast_to(np.asarray(bv, np.float32), (128, D))
    )
    if version == 6:
        return _prep_inputs_v6(x, Wq, bq, Wk, bk, Wv, bv, t, n_cores)
    idx = np.arange(128)
    tri = np.where(idx[:, None] > idx[None, :], np.float32(NEG), np.float32(0.0))
    full = np.full((128, 128), NEG, np.float32)
    zero = np.zeros((128, 128), np.float32)
    if version == 1:
        masks = [
            np.ascontiguousarray(np.concatenate([tri, full], axis=1)),
            np.ascontiguousarray(np.concatenate([zero, tri], axis=1)),
        ]
    elif version == 2:
        masks = []
        for h in range(2):
            m = np.empty((8, 128, 512), np.float32)
            for s_rel in range(8):
                for u in range(4):
                    blk = full if s_rel > 2 * u + h else (tri if s_rel == 2 * u + h else zero)
                    m[s_rel, :, u * 128 : (u + 1) * 128] = blk
            masks.append(np.ascontiguousarray(m))
    else:
        # v3/v4: multiplicative 0/1 bf16 masks, one 128-block per diag s_rel.
        # s_rel even -> block u0=s_rel/2: h=0 diag (keep s<=q), h=1 keep-all
        # s_rel odd  -> block u0:         h=0 dead (zeros),     h=1 diag
        tri01 = (idx[:, None] <= idx[None, :]).astype(NPBF16)
        ones = np.ones((128, 128), NPBF16)
        zeros = np.zeros((128, 128), NPBF16)
        masks = []
        for h in range(2):
            m = np.empty((8, 128, 128), NPBF16)
            for s_rel in range(8):
                if s_rel % 2 == 0:
                    m[s_rel] = tri01 if h == 0 else ones
                else:
                    m[s_rel] = zeros if h == 0 else tri01
            masks.append(m)
        if version >= 4:
            # v4: bk dropped (softmax-invariant) -> cf [128, 258] f32
            cf = np.empty((128, 258), np.float32)
            cf[:, 0:2] = np.asarray(bq, np.float32).reshape(2, 128).T
            cf[:, 2:258] = np.broadcast_to(np.asarray(bv, np.float32), (128, D))
        else:
            # pack constants: cw [128, 1536] bf16, cf [128, 260] f32
            cf = np.empty((128, 260), np.float32)
            cf[:, 0:2] = np.asarray(bq, np.float32).reshape(2, 128).T
            cf[:, 2:4] = np.asarray(bk, np.float32).reshape(2, 128).T
            cf[:, 4:260] = np.broadcast_to(np.asarray(bv, np.float32), (128, D))
        cw = np.empty((128, 1536), NPBF16)
        for j, w in enumerate((Wq, Wk, Wv)):
            wb = np.asarray(w, np.float32).astype(NPBF16).reshape(2, 128, D)
            cw[:, j * 512 : j * 512 + 256] = wb[0]
            cw[:, j * 512 + 256 : j * 512 + 512] = wb[1]
        cms = []
        for h in range(2):
            cm = np.empty((128, 1024), NPBF16)
            for r in range(8):
                cm[:, r * 128 : (r + 1) * 128] = masks[h][r]
            cms.append(np.ascontiguousarray(cm))
        if version >= 4:
            # v4: weights+masks in one bf16 tensor; x/xq as [128, k, cols]
            cwms = [
                np.ascontiguousarray(np.concatenate([cw, cms[h]], axis=1))
                for h in range(2)
            ]
        shared = {"cf": np.ascontiguousarray(cf), "cw": np.ascontiguousarray(cw)}
    in_maps = []
    for c in range(n_cores):
        b, h = divmod(c, 2)
        xb = x[b % b_dim]  # [t, D]
        xT = np.ascontiguousarray(xb.T.astype(NPBF16).reshape(2, 128, t))
        qrows = np.concatenate(
            [xb[g * 128 : (g + 1) * 128] for g in _qtiles(nq, h, version)], axis=0
        )
        xqT = np.ascontiguousarray(qrows.T.astype(NPBF16).reshape(2, 128, tq))
        if version >= 4:
            in_maps.append({
                "xT": np.ascontiguousarray(xT.transpose(1, 0, 2)),
                "xqT": np.ascontiguousarray(xqT.transpose(1, 0, 2)),
                "cwm": cwms[h],
                "cf": shared["cf"],
            })
        elif version == 3:
            in_maps.append({"xT": xT, "xqT": xqT, "cm": cms[h], **shared})
        else:
            in_maps.append({"xT": xT, "xqT": xqT, "mask": masks[h], **shared})
    return in_maps


NPFP8 = ml_dtypes.float8_e4m3fn


def _prep_inputs_v6(x, Wq, bq, Wk, bk, Wv, bv, t: int, n_cores: int):
    """Host prep for v6 (QK-fold). Weight-only folds on host:
    M = Wq Wk^T, c = Wk bq; per-batch thin matvec beta = x c (f32),
    pre-scaled by 1/sqrt(D) for the exp bias operand. x is shipped in
    bf16 (V proj) AND as an fp8 copy (stationary side of S^T)."""
    x = np.asarray(x, dtype=np.float32)
    b_dim = x.shape[0]
    tq = t // 2
    nq = tq // 128
    scale = np.float32(1.0 / np.sqrt(np.float32(D)))

    M = (np.asarray(Wq, np.float32) @ np.asarray(Wk, np.float32).T).astype(NPBF16)
    c = np.asarray(Wk, np.float32) @ np.asarray(bq, np.float32)

    # masks: multiplicative 0/1 bf16, one 128-block per diagonal s_rel
    idx = np.arange(128)
    tri01 = (idx[:, None] <= idx[None, :]).astype(NPBF16)
    ones = np.ones((128, 128), NPBF16)
    zeros = np.zeros((128, 128), NPBF16)
    cwms = []
    for h in range(2):
        cwm = np.zeros((128, 2048), NPBF16)
        cwm[:, 0:256] = M[0:128]
        cwm[:, 256:512] = M[128:256]
        wvb = np.asarray(Wv, np.float32).astype(NPBF16)
        cwm[:, 512:768] = wvb[0:128]
        cwm[:, 768:1024] = wvb[128:256]
        for s_rel in range(8):
            if s_rel % 2 == 0:
                blk = tri01 if h == 0 else ones
            else:
                blk = zeros if h == 0 else tri01
            cwm[:, 1024 + s_rel * 128 : 1024 + (s_rel + 1) * 128] = blk
        cwms.append(np.ascontiguousarray(cwm))

    in_maps = []
    for cid in range(n_cores):
        b, h = divmod(cid, 2)
        xb = x[b % b_dim]  # [t, D] f32
        beta = (xb @ c).astype(np.float32) * scale  # [t]
        cf = np.zeros((128, 288), np.float32)
        cf[:, 0:32] = beta.reshape(32, 128).T
        cf[:, 32:288] = np.broadcast_to(np.asarray(bv, np.float32), (128, D))
        xTb = xb.T.astype(NPBF16).reshape(2, 128, t)  # [k, 128, t]
        xT = np.ascontiguousarray(xTb.transpose(1, 0, 2))
        x8T = np.ascontiguousarray(xTb.astype(NPFP8).transpose(1, 0, 2))
        qrows = np.concatenate(
            [xb[g * 128 : (g + 1) * 128] for g in _qtiles(nq, h, 6)], axis=0
        )
        xqT = np.ascontiguousarray(
            qrows.T.astype(NPBF16).reshape(2, 128, tq).transpose(1, 0, 2)
        )
        in_maps.append({
            "xT": xT,
            "x8T": x8T,
            "xqT": xqT,
            "cwm": cwms[h],
            "cf": np.ascontiguousarray(cf),
        })
    return in_maps


def _qtiles(nq: int, h: int, version: int) -> list[int]:
    """Global q-tile index for each local tile, in local order."""
    if version == 1:
        return [2 * i + h for i in range(nq)]
    return [8 * g + 2 * u + h for g in range(nq // 4) for u in range(4)]


_BUILDERS = {1: build_nc, 2: build_nc_v2, 3: build_nc_v3, 4: build_nc_v4, 5: build_nc_v5, 6: build_nc_v6}


def gather_output(results, t: int = T, n_cores: int = N_CORES, version: int = 1):
    tq = t // 2
    nq = tq // 128
    y = np.empty((n_cores // 2, t, D), np.float32)
    for c in range(n_cores):
        b, h = divmod(c, 2)
        yc = np.asarray(results[c]["y"])
        for li, g in enumerate(_qtiles(nq, h, version)):
            y[b, g * 128 : (g + 1) * 128] = yc[li * 128 : (li + 1) * 128]
    return y


VERSION = 6


def run_on_hw(inputs: dict, trace: bool = False):
    """Returns (y [B,T,D] f32, BassKernelResults)."""
    in_maps = prep_inputs(**inputs, version=VERSION)
    nc = _BUILDERS[VERSION]()
    if not nc.is_finalized():
        nc.finalize()
    res = run_bass_kernel_spmd(nc, in_maps, list(range(N_CORES)), trace=trace)
    return gather_output(res.results, version=VERSION), res


def kernel(**inputs) -> np.ndarray:
    y, _ = run_on_hw(inputs, trace=False)
    return y



# revision 14
# speedup vs baseline: 1.0961x; 1.0122x over previous
"""Causal attention (B=4, T=4096, D=256) on 8 TRN2 NeuronCores.

Sharding: data-parallel over batch x query-halves. Core c handles batch
b = c//2 and query half h = c%2. The active builder (v3, VERSION=3)
groups queries 512 wide: group g of core h owns the interleaved global
128-row query tiles {8g + 2u + h : u in 0..3}, so both halves see the
same s-extent (8g+8 tiles) per group. That makes causal work exactly
balanced AND the program SPMD-uniform: every core runs the identical
instruction stream; only the input DATA (gathered query rows, per-core
0/1 mask blocks) differs.

On-chip layout (flash-attention style, nothing T^2-sized touches HBM):
  xT  [d, t]  (bf16)  -> QT/KT projections directly in transposed layout
                         (lhsT = W [din, dout], rhs = xT)
  S^T [s, q] = matmul(lhsT=KT_tile, rhs=QT_[512 q cols])  (contract d)
  P^T = exp(scale * S^T)  on ACT (no max-subtraction: logits are O(1),
        exp cannot overflow in fp32), then a multiplicative 0/1 bf16
        mask on the single diagonal 128-col block (DVE)
  O   [q, d] = sum_s matmul(lhsT=P^T 128-col slice, rhs=V_ext)
where V_ext has a ones column appended, so O[:, D] accumulates the
softmax denominator for free; final divide is a per-partition scalar.
S^T/exp are narrowed to live columns in the diagonal region and PV
matmuls that are dead on BOTH cores are skipped (both SPMD-uniform).
A short garbage-matmul warm-up bridges the input-DMA wait so the PE
HAM clock gate is at 8/8 when real work arrives, and the single sync
HWDGE queue streams inputs in consumption order (weights, first x
chunks, rest, masks last). V is stored bias-free (bv is added in the
finalize: O/den + bv == using V+bv, since the P rows sum to den), so
the V PSUM->SBUF cast-copies run on the otherwise-idle ACT engine.
Measured ~113-116 us on hardware (8 cores, run-to-run drift ~2 us),
rel err ~2.3e-3 vs the fp32 reference (bf16 matmul precision).
"""

import os
import sys

import numpy as np

for _p in ("/opt/trn_rl_repo", "/root/.axon_site/_ro/trn_rl_repo"):
    if os.path.isdir(_p) and _p not in sys.path:
        sys.path.insert(0, _p)

import ml_dtypes  # noqa: E402

import concourse.bass as bass  # noqa: E402
import concourse.bacc as bacc  # noqa: E402
import concourse.mybir as mybir  # noqa: E402
import concourse.tile as tile  # noqa: E402
from concourse.bass_utils import run_bass_kernel_spmd  # noqa: E402

BF16 = mybir.dt.bfloat16
F32 = mybir.dt.float32
NPBF16 = ml_dtypes.bfloat16

B = 4
T = 4096
D = 256
N_CORES = 8
TQ = T // 2  # query rows per core
NEG = -1.0e9


def build_nc(t: int = T, tq: int = TQ) -> bass.Bass:
    nq = tq // 128  # query tiles per core
    ns = t // 128  # total key tiles
    assert t == 2 * tq and ns == 2 * nq
    scale = 1.0 / float(np.sqrt(np.float32(D)))

    nc = bacc.Bacc()
    xT_d = nc.dram_tensor("xT", [2, 128, t], BF16, kind="ExternalInput")
    xqT_d = nc.dram_tensor("xqT", [2, 128, tq], BF16, kind="ExternalInput")
    wq_d = nc.dram_tensor("wq", [2, 128, D], BF16, kind="ExternalInput")
    wk_d = nc.dram_tensor("wk", [2, 128, D], BF16, kind="ExternalInput")
    wv_d = nc.dram_tensor("wv", [2, 128, D], BF16, kind="ExternalInput")
    bq_d = nc.dram_tensor("bq", [2, 128, 1], F32, kind="ExternalInput")
    bk_d = nc.dram_tensor("bk", [2, 128, 1], F32, kind="ExternalInput")
    bvb_d = nc.dram_tensor("bvb", [128, D], F32, kind="ExternalInput")
    mask_d = nc.dram_tensor("mask", [128, 256], F32, kind="ExternalInput")
    y_d = nc.dram_tensor("y", [tq, D], F32, kind="ExternalOutput")

    with tile.TileContext(nc) as tc:
        with (
            tc.tile_pool(name="persist", bufs=1) as pp,
            tc.tile_pool(name="vpool", bufs=1) as vp,
            tc.tile_pool(name="pj_ps", bufs=2, space="PSUM") as pj_ps,
            tc.tile_pool(name="st_ps", bufs=3, space="PSUM") as st_ps,
            tc.tile_pool(name="o_ps", bufs=2, space="PSUM") as o_ps_pool,
            tc.tile_pool(name="ptp", bufs=4) as ptp,
            tc.tile_pool(name="outp", bufs=3) as outp,
            tc.tile_pool(name="finp", bufs=3) as finp,
        ):
            # ---- persistent SBUF inputs
            xT = [pp.tile([128, t], BF16, name=f"xT{k}") for k in range(2)]
            xqT = [pp.tile([128, tq], BF16, name=f"xqT{k}") for k in range(2)]
            wq = [pp.tile([128, D], BF16, name=f"wq{k}") for k in range(2)]
            wk = [pp.tile([128, D], BF16, name=f"wk{k}") for k in range(2)]
            wv = [pp.tile([128, D], BF16, name=f"wv{k}") for k in range(2)]
            bq = [pp.tile([128, 1], F32, name=f"bq{k}") for k in range(2)]
            bk = [pp.tile([128, 1], F32, name=f"bk{k}") for k in range(2)]
            bvb = pp.tile([128, D], F32, name="bvb")
            mask = pp.tile([128, 256], F32, name="mask")
            for k in range(2):
                nc.sync.dma_start(xT[k][:], xT_d[k])
                nc.sync.dma_start(xqT[k][:], xqT_d[k])
                nc.sync.dma_start(wq[k][:], wq_d[k])
                nc.sync.dma_start(wk[k][:], wk_d[k])
                nc.sync.dma_start(wv[k][:], wv_d[k])
                nc.sync.dma_start(bq[k][:], bq_d[k])
                nc.sync.dma_start(bk[k][:], bk_d[k])
            nc.sync.dma_start(bvb[:], bvb_d[:])
            nc.sync.dma_start(mask[:], mask_d[:])

            # ---- projections: KT/QT in [dout, t] layout (bias via DVE)
            KT = [pp.tile([128, t], BF16, name=f"KT{m}") for m in range(2)]
            QT = [pp.tile([128, tq], BF16, name=f"QT{m}") for m in range(2)]
            NBK = min(512, t)
            NBQ = min(512, tq)
            for m in range(2):
                ms = slice(m * 128, (m + 1) * 128)
                for nb in range(t // NBK):
                    ps = pj_ps.tile([128, NBK], F32, name="pj", tag="pj")
                    for k in range(2):
                        nc.tensor.matmul(
                            ps[:],
                            wk[k][:, ms],
                            xT[k][:, nb * NBK : (nb + 1) * NBK],
                            start=(k == 0),
                            stop=(k == 1),
                        )
                    nc.vector.tensor_scalar_add(
                        KT[m][:, nb * NBK : (nb + 1) * NBK], ps[:], bk[m][:]
                    )
                for nb in range(tq // NBQ):
                    ps = pj_ps.tile([128, NBQ], F32, name="pj", tag="pj")
                    for k in range(2):
                        nc.tensor.matmul(
                            ps[:],
                            wq[k][:, ms],
                            xqT[k][:, nb * NBQ : (nb + 1) * NBQ],
                            start=(k == 0),
                            stop=(k == 1),
                        )
                    nc.vector.tensor_scalar_add(
                        QT[m][:, nb * NBQ : (nb + 1) * NBQ], ps[:], bq[m][:]
                    )

            # ---- V projection: natural [s, d] layout + ones column
            V = [vp.tile([128, D + 1], BF16, name=f"v{s}") for s in range(ns)]
            for s in range(ns):
                ps = pj_ps.tile([128, D], F32, name="pj", tag="pj")
                for k in range(2):
                    nc.tensor.matmul(
                        ps[:],
                        xT[k][:, s * 128 : (s + 1) * 128],
                        wv[k][:],
                        start=(k == 0),
                        stop=(k == 1),
                    )
                nc.vector.tensor_add(V[s][:, 0:D], ps[:], bvb[:])
                nc.vector.memset(V[s][:, D : D + 1], 1.0)

            # ---- attention
            exp_t = mybir.ActivationFunctionType.Exp
            for i in range(nq):
                e = 2 * i + 2  # s-tiles this query tile touches
                o_ps = o_ps_pool.tile([128, D + 1], F32, name="ops", tag="ops")
                qs = slice(i * 128, (i + 1) * 128)
                for s in range(e):
                    stp = st_ps.tile([128, 128], F32, name="stp", tag="stp")
                    for k in range(2):
                        nc.tensor.matmul(
                            stp[:],
                            KT[k][:, s * 128 : (s + 1) * 128],
                            QT[k][:, qs],
                            start=(k == 0),
                            stop=(k == 1),
                        )
                    if s == e - 2:
                        nc.vector.tensor_add(stp[:], stp[:], mask[:, 0:128])
                    elif s == e - 1:
                        nc.vector.tensor_add(stp[:], stp[:], mask[:, 128:256])
                    pt = ptp.tile([128, 128], BF16, name="pt", tag="pt")
                    nc.scalar.activation(pt[:], stp[:], exp_t, scale=scale)
                    nc.tensor.matmul(
                        o_ps[:], pt[:], V[s][:], start=(s == 0), stop=(s == e - 1)
                    )
                rec = finp.tile([128, 1], F32, name="rec", tag="rec")
                nc.vector.reciprocal(rec[:], o_ps[:, D : D + 1])
                ob = outp.tile([128, D], F32, name="ob", tag="ob")
                nc.vector.tensor_scalar_mul(ob[:], o_ps[:, 0:D], rec[:])
                nc.sync.dma_start(y_d[i * 128 : (i + 1) * 128, :], ob[:])
    return nc


def build_nc_v2(t: int = T, tq: int = TQ) -> bass.Bass:
    """Quad-grouped attention: 4 query tiles (512 q cols) share each S^T
    matmul / exp pass. Core h owns global q-tiles {8g + 2u + h}; group g
    runs a uniform s-extent of 8g+8 tiles on every core."""
    nq = tq // 128
    ns = t // 128
    ng = nq // 4
    assert t == 2 * tq and nq % 4 == 0
    scale = 1.0 / float(np.sqrt(np.float32(D)))

    nc = bacc.Bacc()
    xT_d = nc.dram_tensor("xT", [2, 128, t], BF16, kind="ExternalInput")
    xqT_d = nc.dram_tensor("xqT", [2, 128, tq], BF16, kind="ExternalInput")
    wq_d = nc.dram_tensor("wq", [2, 128, D], BF16, kind="ExternalInput")
    wk_d = nc.dram_tensor("wk", [2, 128, D], BF16, kind="ExternalInput")
    wv_d = nc.dram_tensor("wv", [2, 128, D], BF16, kind="ExternalInput")
    bq_d = nc.dram_tensor("bq", [2, 128, 1], F32, kind="ExternalInput")
    bk_d = nc.dram_tensor("bk", [2, 128, 1], F32, kind="ExternalInput")
    bvb_d = nc.dram_tensor("bvb", [128, D], F32, kind="ExternalInput")
    mask_d = nc.dram_tensor("mask", [8, 128, 512], F32, kind="ExternalInput")
    y_d = nc.dram_tensor("y", [tq, D], F32, kind="ExternalOutput")

    with tile.TileContext(nc) as tc:
        with (
            tc.tile_pool(name="persist", bufs=1) as pp,
            tc.tile_pool(name="vpool", bufs=1) as vp,
            tc.tile_pool(name="st_ps", bufs=2, space="PSUM") as st_ps,
            tc.tile_pool(name="o_ps", bufs=1, space="PSUM") as o_ps_pool,
            tc.tile_pool(name="ptp", bufs=3) as ptp,
            tc.tile_pool(name="outp", bufs=3) as outp,
            tc.tile_pool(name="finp", bufs=3) as finp,
        ):
            # ---- persistent SBUF inputs
            xT = [pp.tile([128, t], BF16, name=f"xT{k}") for k in range(2)]
            xqT = [pp.tile([128, tq], BF16, name=f"xqT{k}") for k in range(2)]
            wq = [pp.tile([128, D], BF16, name=f"wq{k}") for k in range(2)]
            wk = [pp.tile([128, D], BF16, name=f"wk{k}") for k in range(2)]
            wv = [pp.tile([128, D], BF16, name=f"wv{k}") for k in range(2)]
            bq = [pp.tile([128, 1], F32, name=f"bq{k}") for k in range(2)]
            bk = [pp.tile([128, 1], F32, name=f"bk{k}") for k in range(2)]
            bvb = pp.tile([128, D], F32, name="bvb")
            mask = [pp.tile([128, 512], F32, name=f"mask{r}") for r in range(8)]
            for k in range(2):
                nc.sync.dma_start(xT[k][:], xT_d[k])
                nc.sync.dma_start(xqT[k][:], xqT_d[k])
                nc.sync.dma_start(wq[k][:], wq_d[k])
                nc.sync.dma_start(wk[k][:], wk_d[k])
                nc.sync.dma_start(wv[k][:], wv_d[k])
                nc.sync.dma_start(bq[k][:], bq_d[k])
                nc.sync.dma_start(bk[k][:], bk_d[k])
            nc.sync.dma_start(bvb[:], bvb_d[:])
            for r in range(8):
                nc.sync.dma_start(mask[r][:], mask_d[r])

            KT = [pp.tile([128, t], BF16, name=f"KT{m}") for m in range(2)]
            QT = [pp.tile([128, tq], BF16, name=f"QT{m}") for m in range(2)]
            V = [vp.tile([128, D + 1], BF16, name=f"v{s}") for s in range(ns)]

            # ---- projections in their own PSUM pool (freed before attention)
            with tc.tile_pool(name="pj_ps", bufs=2, space="PSUM") as pj_ps:
                NBK = min(512, t)
                NBQ = min(512, tq)
                for m in range(2):
                    ms = slice(m * 128, (m + 1) * 128)
                    for nb in range(t // NBK):
                        ps = pj_ps.tile([128, NBK], F32, name="pj", tag="pj")
                        for k in range(2):
                            nc.tensor.matmul(
                                ps[:],
                                wk[k][:, ms],
                                xT[k][:, nb * NBK : (nb + 1) * NBK],
                                start=(k == 0),
                                stop=(k == 1),
                            )
                        nc.vector.tensor_scalar_add(
                            KT[m][:, nb * NBK : (nb + 1) * NBK], ps[:], bk[m][:]
                        )
                    for nb in range(tq // NBQ):
                        ps = pj_ps.tile([128, NBQ], F32, name="pj", tag="pj")
                        for k in range(2):
                            nc.tensor.matmul(
                                ps[:],
                                wq[k][:, ms],
                                xqT[k][:, nb * NBQ : (nb + 1) * NBQ],
                                start=(k == 0),
                                stop=(k == 1),
                            )
                        nc.vector.tensor_scalar_add(
                            QT[m][:, nb * NBQ : (nb + 1) * NBQ], ps[:], bq[m][:]
                        )
                for s in range(ns):
                    ps = pj_ps.tile([128, D], F32, name="pj", tag="pj")
                    for k in range(2):
                        nc.tensor.matmul(
                            ps[:],
                            xT[k][:, s * 128 : (s + 1) * 128],
                            wv[k][:],
                            start=(k == 0),
                            stop=(k == 1),
                        )
                    nc.vector.tensor_add(V[s][:, 0:D], ps[:], bvb[:])
                    nc.vector.memset(V[s][:, D : D + 1], 1.0)

            # ---- attention, 512 q cols per group
            exp_t = mybir.ActivationFunctionType.Exp
            att = ctx_att = tc.tile_pool(name="st_ps", bufs=2, space="PSUM")
            st_ps = att.__enter__()
            o_ctx = tc.tile_pool(name="o_ps", bufs=6, space="PSUM")
            o_ps_pool = o_ctx.__enter__()
            for g in range(ng):
                e = 8 * g + 8
                qs = slice(g * 512, (g + 1) * 512)
                o = [
                    o_ps_pool.tile([128, D + 1], F32, name=f"o{u}", tag=f"o{u}")
                    for u in range(4)
                ]
                for s in range(e):
                    stp = st_ps.tile([128, 512], F32, name="stp", tag="stp")
                    for k in range(2):
                        nc.tensor.matmul(
                            stp[:],
                            KT[k][:, s * 128 : (s + 1) * 128],
                            QT[k][:, qs],
                            start=(k == 0),
                            stop=(k == 1),
                        )
                    if s >= 8 * g:
                        nc.vector.tensor_add(stp[:], stp[:], mask[s - 8 * g][:])
                    pt = ptp.tile([128, 512], BF16, name="pt", tag="pt")
                    nc.scalar.activation(pt[:], stp[:], exp_t, scale=scale)
                    for u in range(4):
                        nc.tensor.matmul(
                            o[u][:],
                            pt[:, u * 128 : (u + 1) * 128],
                            V[s][:],
                            start=(s == 0),
                            stop=(s == e - 1),
                        )
                for u in range(4):
                    rec = finp.tile([128, 1], F32, name="rec", tag="rec")
                    nc.vector.reciprocal(rec[:], o[u][:, D : D + 1])
                    ob = outp.tile([128, D], F32, name="ob", tag="ob")
                    nc.vector.tensor_scalar_mul(ob[:], o[u][:, 0:D], rec[:])
                    lrow = (g * 4 + u) * 128
                    nc.sync.dma_start(y_d[lrow : lrow + 128, :], ob[:])
    return nc


def build_nc_v3(t: int = T, tq: int = TQ, st_bufs: int = 4, o_bufs: int = 4, pt_bufs: int = 6, pj_bufs: int = 4) -> bass.Bass:
    """v2 + cheaper masking, less dead work, and walrus-friendly syncs:
    - all constants (weights, biases, masks) packed into two DRAM tensors
      loaded with one DMA each; tiny DVE "absorber" copies pull the DMA
      completion into DVE's vector clock so the bias TensorScalarPtr ops
      carry a single sem wait (walrus rejects multi-wait TS instrs);
    - causal mask applied AFTER exp as a multiplicative 0/1 bf16 mask on
      one 128-col block per diagonal s-tile (DVE bf16 SBUF fast mode);
    - PV matmuls skipped for (s_rel, u) tiles dead on BOTH cores
      (u < floor(s_rel/2)) — the skip pattern is SPMD-uniform;
    - input x DMAs chunked so projections overlap the loads;
    - single-tag o-pool (bufs=6) so group g+1 does not wait on group g's
      finalize."""
    nq = tq // 128
    ns = t // 128
    ng = nq // 4
    assert t == 2 * tq and nq % 4 == 0
    scale = 1.0 / float(np.sqrt(np.float32(D)))

    nc = bacc.Bacc()
    xT_d = nc.dram_tensor("xT", [2, 128, t], BF16, kind="ExternalInput")
    xqT_d = nc.dram_tensor("xqT", [2, 128, tq], BF16, kind="ExternalInput")
    cw_d = nc.dram_tensor("cw", [128, 1536], BF16, kind="ExternalInput")
    cm_d = nc.dram_tensor("cm", [128, 1024], BF16, kind="ExternalInput")
    cf_d = nc.dram_tensor("cf", [128, 260], F32, kind="ExternalInput")
    y_d = nc.dram_tensor("y", [tq, D], F32, kind="ExternalOutput")

    with tile.TileContext(nc) as tc:
        with (
            tc.tile_pool(name="persist", bufs=1) as pp,
            tc.tile_pool(name="vpool", bufs=1) as vp,
            tc.tile_pool(name="ptp", bufs=pt_bufs) as ptp,
            tc.tile_pool(name="outp", bufs=3) as outp,
            tc.tile_pool(name="finp", bufs=4) as finp,
        ):
            # ---- inputs. One sync (HWDGE) queue so transfers complete in
            # priority order: weights -> first x chunks (gates the first
            # projection matmuls) -> rest -> masks (needed ~20us in).
            cw = pp.tile([128, 1536], BF16, name="cw")
            cm = pp.tile([128, 1024], BF16, name="cm")
            cf = pp.tile([128, 260], F32, name="cf")
            xT = [pp.tile([128, t], BF16, name=f"xT{k}") for k in range(2)]
            xqT = [pp.tile([128, tq], BF16, name=f"xqT{k}") for k in range(2)]
            CH = max(512, t // 2)
            nc.sync.dma_start(cw[:], cw_d[:])
            for k in range(2):
                nc.sync.dma_start(xT[k][:, 0:CH], xT_d[k][:, 0:CH])
            nc.sync.dma_start(cf[:], cf_d[:])
            for c0 in range(CH, t, CH):
                for k in range(2):
                    nc.sync.dma_start(xT[k][:, c0 : c0 + CH], xT_d[k][:, c0 : c0 + CH])
            for k in range(2):
                nc.sync.dma_start(xqT[k][:], xqT_d[k])
            nc.sync.dma_start(cm[:], cm_d[:])
            # absorber copies: pull each const DMA's completion into DVE's
            # vector clock so downstream DVE ops carry a single sem wait
            scrb = finp.tile([128, 1], BF16, name="scrb", tag="scrb")
            nc.vector.tensor_copy(scrb[:], cw[:, 0:1])
            scrf = finp.tile([128, 1], F32, name="scrf", tag="scrf")
            nc.vector.tensor_copy(scrf[:], cf[:, 0:1])
            scrm = finp.tile([128, 1], BF16, name="scrm", tag="scrm")
            nc.vector.tensor_copy(scrm[:], cm[:, 0:1])
            wq = [cw[:, 0 + k * 256 : 256 + k * 256] for k in range(2)]
            wk = [cw[:, 512 + k * 256 : 768 + k * 256] for k in range(2)]
            wv = [cw[:, 1024 + k * 256 : 1280 + k * 256] for k in range(2)]
            maskb = [cm[:, r * 128 : (r + 1) * 128] for r in range(8)]
            bq = [cf[:, k : k + 1] for k in range(2)]
            bk = [cf[:, 2 + k : 3 + k] for k in range(2)]
            bvb = cf[:, 4:260]

            # HAM warm-up: garbage matmuls while input DMAs land, so the
            # PE clock gate is already at 8/8 when real work arrives.
            wa = pp.tile([128, 128], BF16, name="wa")
            wb = pp.tile([128, 512], BF16, name="wb")
            nc.vector.memset(wa[:], 0.0)
            nc.vector.memset(wb[:], 0.0)
            with tc.tile_pool(name="warm_ps", bufs=1, space="PSUM") as wps:
                wp_t = wps.tile([128, 512], F32, name="warm")
                for _ in range(20):
                    nc.tensor.matmul(wp_t[:], wa[:], wb[:], start=True, stop=True)

            KT = [pp.tile([128, t], BF16, name=f"KT{m}") for m in range(2)]
            QT = [pp.tile([128, tq], BF16, name=f"QT{m}") for m in range(2)]
            V = [vp.tile([128, D + 1], BF16, name=f"v{s}") for s in range(ns)]

            # ---- projections in their own PSUM pool (freed before attention)
            with tc.tile_pool(name="pj_ps", bufs=pj_bufs, space="PSUM") as pj_ps:
                NBK = min(512, t)
                NBQ = min(512, tq)
                for nb in range(t // NBK):
                    for m in range(2):
                        ms = slice(m * 128, (m + 1) * 128)
                        ps = pj_ps.tile([128, NBK], F32, name="pj", tag="pj")
                        for k in range(2):
                            nc.tensor.matmul(
                                ps[:],
                                wk[k][:, ms],
                                xT[k][:, nb * NBK : (nb + 1) * NBK],
                                start=(k == 0),
                                stop=(k == 1),
                            )
                        nc.vector.tensor_scalar_add(
                            KT[m][:, nb * NBK : (nb + 1) * NBK], ps[:], bk[m]
                        )
                for m in range(2):
                    ms = slice(m * 128, (m + 1) * 128)
                    for nb in range(tq // NBQ):
                        ps = pj_ps.tile([128, NBQ], F32, name="pj", tag="pj")
                        for k in range(2):
                            nc.tensor.matmul(
                                ps[:],
                                wq[k][:, ms],
                                xqT[k][:, nb * NBQ : (nb + 1) * NBQ],
                                start=(k == 0),
                                stop=(k == 1),
                            )
                        nc.vector.tensor_scalar_add(
                            QT[m][:, nb * NBQ : (nb + 1) * NBQ], ps[:], bq[m]
                        )

            # ---- attention, 512 q cols per group
            exp_t = mybir.ActivationFunctionType.Exp
            att = ctx_att = tc.tile_pool(name="st_ps", bufs=st_bufs, space="PSUM")
            st_ps = att.__enter__()
            o_ctx = tc.tile_pool(name="o_ps", bufs=o_bufs, space="PSUM")
            o_ps_pool = o_ctx.__enter__()
            for g in range(ng):
                e = 8 * g + 8
                for s in range(8 * g, min(8 * g + 8, ns)):
                    ps = st_ps.tile([128, D], F32, name="vpj", tag="stp")
                    for k in range(2):
                        nc.tensor.matmul(
                            ps[:],
                            xT[k][:, s * 128 : (s + 1) * 128],
                            wv[k],
                            start=(k == 0),
                            stop=(k == 1),
                        )
                    # bias-free V: since sum_s P = den, (O + den*bv)/den =
                    # O/den + bv, so bv moves to the finalize and this
                    # PSUM->SBUF cast-copy runs on the idle ACT engine
                    nc.scalar.copy(V[s][:, 0:D], ps[:])
                    nc.vector.memset(V[s][:, D : D + 1], 1.0)
                qs = slice(g * 512, (g + 1) * 512)
                o = [
                    o_ps_pool.tile([128, D + 1], F32, name=f"o{u}", tag="o")
                    for u in range(4)
                ]
                for s in range(e):
                    s_rel = s - 8 * g
                    u0 = max(s_rel, 0) // 2  # first live 128-col block
                    c0 = u0 * 128
                    stp = st_ps.tile([128, 512], F32, name="stp", tag="stp")
                    for k in range(2):
                        nc.tensor.matmul(
                            stp[:, c0:512],
                            KT[k][:, s * 128 : (s + 1) * 128],
                            QT[k][:, g * 512 + c0 : (g + 1) * 512],
                            start=(k == 0),
                            stop=(k == 1),
                        )
                    pt = ptp.tile([128, 512], BF16, name="pt", tag="pt")
                    nc.scalar.activation(
                        pt[:, c0:512], stp[:, c0:512], exp_t, scale=scale
                    )
                    if s_rel >= 0:
                        nc.vector.tensor_mul(
                            pt[:, c0 : c0 + 128],
                            pt[:, c0 : c0 + 128],
                            maskb[s_rel],
                        )
                    for u in range(4):
                        if s_rel >= 0 and u < s_rel // 2:
                            continue  # dead on every core
                        nc.tensor.matmul(
                            o[u][:],
                            pt[:, u * 128 : (u + 1) * 128],
                            V[s][:],
                            start=(s == 0),
                            stop=(s == 8 * g + 2 * u + 1),
                        )
                for u in range(4):
                    rec = finp.tile([128, 1], F32, name="rec", tag="rec")
                    nc.vector.reciprocal(rec[:], o[u][:, D : D + 1])
                    ob = outp.tile([128, D], F32, name="ob", tag="ob")
                    nc.vector.scalar_tensor_tensor(
                        ob[:],
                        o[u][:, 0:D],
                        rec[:],
                        bvb,
                        mybir.AluOpType.mult,
                        mybir.AluOpType.add,
                    )
                    lrow = (g * 4 + u) * 128
                    nc.sync.dma_start(y_d[lrow : lrow + 128, :], ob[:])
            o_ctx.__exit__(None, None, None)
            ctx_att.__exit__(None, None, None)
    return nc


def build_nc_v4(
    t: int = T,
    tq: int = TQ,
    st_bufs: int = 3,
    o_bufs: int = 5,
    pt_bufs: int = 6,
    warm_n: int = 12,
) -> bass.Bass:
    """v3 + fp8 S^T and a fully interleaved projection/attention pipeline.

    - S^T = K^T·Q runs as ONE fp8e4 DoubleRow matmul per (s, group): the PE
      contracts all 256 d-rows in a single pass (2 rows/cycle), halving the
      S cost vs the bf16 2-pass version. Q/K are projected in bf16 precision
      (PSUM f32) and only quantized at the PSUM->SBUF cast (measured rel err
      ~1.1e-2 vs fp32 reference, tolerance 2e-2). P and V stay bf16 (fp8
      there costs ~3.6% rms -> fails tolerance).
    - K's bias is dropped entirely: softmax is invariant to the row-constant
      q·bk term, so only bq (via the Q cast) matters.
    - Projections are interleaved per query-group g and allocate their PSUM
      from the SAME pool/tag as the o accumulators (every slot is a full
      bank anyway): the 4-deep rotation hides the ~0.7us cast latency that
      a dedicated 2-buf pool exposed, and st_ps gets 4 banks for deeper S
      lookahead. Casts/copies alternate DVE/ACT to split the backlog.
    - Inputs stream on BOTH hardware DGE queues (SP + ACT); transfers on one
      queue serialize (each push waits the previous transfer's completion
      semaphore), so the host packs x/xq/consts into few, need-ordered
      transfers: [cwm | xq-g0 | cf | xq-rest] on ACT, 4 x group-chunks on SP.
    - o[u] finalize (reciprocal + scale + bias + output DMA) is emitted
      inside the s-loop right after u's stopping matmul, so the tail after
      the last PV matmul is just one finalize chain.
    """
    nq = tq // 128
    ns = t // 128
    ng = nq // 4
    assert t == 2 * tq and nq % 4 == 0
    scale = 1.0 / float(np.sqrt(np.float32(D)))
    FP8 = mybir.dt.float8e4

    nc = bacc.Bacc()
    xT_d = nc.dram_tensor("xT", [128, 2, t], BF16, kind="ExternalInput")
    xqT_d = nc.dram_tensor("xqT", [128, 2, tq], BF16, kind="ExternalInput")
    cwm_d = nc.dram_tensor("cwm", [128, 2560], BF16, kind="ExternalInput")
    cf_d = nc.dram_tensor("cf", [128, 258], F32, kind="ExternalInput")
    y_d = nc.dram_tensor("y", [tq, D], F32, kind="ExternalOutput")

    with tile.TileContext(nc) as tc:
        with (
            tc.tile_pool(name="persist", bufs=1) as pp,
            tc.tile_pool(name="vpool", bufs=1) as vp,
            tc.tile_pool(name="ptp", bufs=pt_bufs) as ptp,
            tc.tile_pool(name="outp", bufs=3) as outp,
            tc.tile_pool(name="finp", bufs=4) as finp,
        ):
            cwm = pp.tile([128, 2560], BF16, name="cwm")
            cf = pp.tile([128, 258], F32, name="cf")
            xTa = pp.tile([128, 2 * t], BF16, name="xTa")
            xqTa = pp.tile([128, 2 * tq], BF16, name="xqTa")
            xTv = xTa[:].rearrange("p (k c) -> p k c", k=2)
            xqTv = xqTa[:].rearrange("p (k c) -> p k c", k=2)
            xT = [xTa[:, k * t : (k + 1) * t] for k in range(2)]
            xqT = [xqTa[:, k * tq : (k + 1) * tq] for k in range(2)]

            def xts(k, a, b):
                return xT[k][:, a:b]

            def xqs(k, a, b):
                return xqT[k][:, a:b]
            KT8 = pp.tile([128, 2 * t], FP8, name="KT8")
            QT8 = pp.tile([128, 2 * tq], FP8, name="QT8")
            KT8v = KT8[:].rearrange("p (m c) -> p m c", m=2)
            QT8v = QT8[:].rearrange("p (m c) -> p m c", m=2)
            V = [vp.tile([128, D + 1], BF16, name=f"v{s}") for s in range(ns)]

            # ---- input DMA: the two DGE queues share ~358GB/s of HBM and
            # the ACT-side queue starts late and runs at ~half rate, so the
            # group-0-critical set streams need-ordered on the fast SP
            # queue; only the latest-needed bulk rides the ACT queue.
            nc.scalar.dma_start(xqTv[:, :, 0:512], xqT_d[:, :, 0:512])
            nc.scalar.dma_start(cf[:], cf_d[:])
            nc.sync.dma_start(cwm[:], cwm_d[:])
            nc.sync.dma_start(xTv[:, :, 0:1024], xT_d[:, :, 0:1024])
            nc.sync.dma_start(xTv[:, :, 1024:2048], xT_d[:, :, 1024:2048])
            nc.sync.dma_start(xqTv[:, :, 512:1024], xqT_d[:, :, 512:1024])
            nc.sync.dma_start(xTv[:, :, 2048:3072], xT_d[:, :, 2048:3072])
            nc.sync.dma_start(xqTv[:, :, 1024:tq], xqT_d[:, :, 1024:tq])
            nc.sync.dma_start(xTv[:, :, 3072:t], xT_d[:, :, 3072:t])

            wq = [cwm[:, 0 + k * 256 : 256 + k * 256] for k in range(2)]
            wk = [cwm[:, 512 + k * 256 : 768 + k * 256] for k in range(2)]
            wv = [cwm[:, 1024 + k * 256 : 1280 + k * 256] for k in range(2)]
            maskb = [cwm[:, 1536 + r * 128 : 1536 + (r + 1) * 128] for r in range(8)]
            bq = [cf[:, k : k + 1] for k in range(2)]
            bvb = cf[:, 2:258]

            # warm-up garbage matmuls bridge the input-DMA wait so the PE
            # HAM clock gate is at 8/8 when real work arrives
            wa = pp.tile([128, 128], BF16, name="wa")
            wb = pp.tile([128, 512], BF16, name="wb")
            nc.vector.memset(wa[:], 0.0)
            nc.vector.memset(wb[:], 0.0)
            with tc.tile_pool(name="warm_ps", bufs=1, space="PSUM") as wps:
                wp_t = wps.tile([128, 512], F32, name="warm")
                for _ in range(warm_n):
                    nc.tensor.matmul(wp_t[:], wa[:], wb[:], start=True, stop=True)

            # denominator ones-column, set once per V tile (off critical path)
            for s in range(ns):
                nc.vector.memset(V[s][:, D : D + 1], 1.0)
            # absorber copies pull const DMA completions into each consumer
            # engine's vector clock (single extra sem wait per consumer op)
            scrb = finp.tile([128, 1], BF16, name="scrb", tag="scrb")
            nc.vector.tensor_copy(scrb[:], cwm[:, 0:1])
            scrf = finp.tile([128, 1], F32, name="scrf", tag="scrf")
            nc.vector.tensor_copy(scrf[:], cf[:, 0:1])
            scrg = finp.tile([128, 1], F32, name="scrg", tag="scrg")
            nc.scalar.copy(scrg[:], cf[:, 0:1])

            exp_t = mybir.ActivationFunctionType.Exp
            id_t = mybir.ActivationFunctionType.Identity
            with (
                tc.tile_pool(name="st_ps", bufs=st_bufs, space="PSUM") as st_ps,
                tc.tile_pool(name="o_ps", bufs=o_bufs, space="PSUM") as o_ps_pool,
            ):
                def q_proj(g):
                    for m in range(2):
                        ms = slice(m * 128, (m + 1) * 128)
                        ps = o_ps_pool.tile([128, 512], F32, name="pj", tag="o")
                        for k in range(2):
                            nc.tensor.matmul(
                                ps[:], wq[k][:, ms],
                                xqs(k, g * 512, (g + 1) * 512),
                                start=(k == 0), stop=(k == 1),
                            )
                        dst = QT8[:, m * tq + g * 512 : m * tq + (g + 1) * 512]
                        if m == 0:
                            nc.vector.tensor_scalar_add(dst, ps[:], bq[m])
                        else:
                            nc.scalar.activation(dst, ps[:], id_t, bias=bq[m])

                def k_proj(g):
                    # no bias: q·bk is row-constant, softmax-invariant
                    for nb in range(2):
                        ca = g * 1024 + nb * 512
                        for m in range(2):
                            ms = slice(m * 128, (m + 1) * 128)
                            ps = o_ps_pool.tile([128, 512], F32, name="pj", tag="o")
                            for k in range(2):
                                nc.tensor.matmul(
                                    ps[:], wk[k][:, ms], xts(k, ca, ca + 512),
                                    start=(k == 0), stop=(k == 1),
                                )
                            dst = KT8[
                                :,
                                m * t + g * 1024 + nb * 512 :
                                m * t + g * 1024 + (nb + 1) * 512,
                            ]
                            if (2 * nb + m) % 2 == 0:
                                nc.vector.tensor_copy(dst, ps[:])
                            else:
                                nc.scalar.copy(dst, ps[:])

                def v_proj(g):
                    # bias-free; bv is added in the finalize
                    for s in range(8 * g, 8 * g + 8):
                        ps = o_ps_pool.tile([128, D], F32, name="vpj", tag="o")
                        for k in range(2):
                            nc.tensor.matmul(
                                ps[:], xts(k, s * 128, (s + 1) * 128), wv[k],
                                start=(k == 0), stop=(k == 1),
                            )
                        if s % 2 == 0:
                            nc.scalar.copy(V[s][:, 0:D], ps[:])
                        else:
                            nc.vector.tensor_copy(V[s][:, 0:D], ps[:])

                for g in range(ng):
                    e = 8 * g + 8
                    q_proj(g), k_proj(g), v_proj(g)
                    if g == 0:
                        scrm = finp.tile([128, 1], BF16, name="scrm", tag="scrm")
                        nc.vector.tensor_copy(scrm[:], cwm[:, 1536:1537])
                    # ---- attention for group g
                    o = [
                        o_ps_pool.tile([128, D + 1], F32, name=f"o{u}", tag="o")
                        for u in range(4)
                    ]
                    for s in range(e):
                        s_rel = s - 8 * g
                        u0 = max(s_rel, 0) // 2
                        c0 = u0 * 128
                        stp = st_ps.tile([128, 512], F32, name="stp", tag="stp")
                        nc.tensor.matmul(
                            stp[:, c0:512],
                            KT8v[:, :, s * 128 : (s + 1) * 128],
                            QT8v[:, :, g * 512 + c0 : (g + 1) * 512],
                            start=True, stop=True,
                            perf_mode=mybir.MatmulPerfMode.DoubleRow,
                        )
                        pt = ptp.tile([128, 512], BF16, name="pt", tag="pt")
                        nc.scalar.activation(
                            pt[:, c0:512], stp[:, c0:512], exp_t, scale=scale
                        )
                        if s_rel >= 0:
                            nc.vector.tensor_mul(
                                pt[:, c0 : c0 + 128],
                                pt[:, c0 : c0 + 128],
                                maskb[s_rel],
                            )
                        # u0's P block waits on the DVE mask -> run it last
                        for u in list(range(u0 + 1, 4)) + [u0]:
                            nc.tensor.matmul(
                                o[u][:],
                                pt[:, u * 128 : (u + 1) * 128],
                                V[s][:],
                                start=(s == 0),
                                stop=(s == 8 * g + 2 * u + 1),
                            )
                            if s == 8 * g + 2 * u + 1:
                                rec = finp.tile([128, 1], F32, name="rec", tag="rec")
                                nc.vector.reciprocal(rec[:], o[u][:, D : D + 1])
                                ob = outp.tile([128, D], F32, name="ob", tag="ob")
                                nc.vector.scalar_tensor_tensor(
                                    ob[:], o[u][:, 0:D], rec[:], bvb,
                                    mybir.AluOpType.mult, mybir.AluOpType.add,
                                )
                                lrow = (g * 4 + u) * 128
                                nc.sync.dma_start(y_d[lrow : lrow + 128, :], ob[:])
    return nc


def build_nc_v5(
    t: int = T,
    tq: int = TQ,
    st_bufs: int = 3,
    o_bufs: int = 5,
    pt_bufs: int = 6,
    warm_n: int = 12,
) -> bass.Bass:
    """v4 + engine rebalance: ACT runs ONLY the 80 exp activations (its
    ~0.83ns/col + ~143ns/op makes it the co-bottleneck in v4 where it also
    carried K/V/Q-bias casts). All projection PSUM->SBUF casts move to DVE
    and Pool (gpsimd):
      - K fp8 casts + Q bias-adds -> DVE
      - V bf16 casts + ones-col memsets -> Pool
    Everything else identical to v4.
    """
    nq = tq // 128
    ns = t // 128
    ng = nq // 4
    assert t == 2 * tq and nq % 4 == 0
    scale = 1.0 / float(np.sqrt(np.float32(D)))
    FP8 = mybir.dt.float8e4

    nc = bacc.Bacc()
    xT_d = nc.dram_tensor("xT", [128, 2, t], BF16, kind="ExternalInput")
    xqT_d = nc.dram_tensor("xqT", [128, 2, tq], BF16, kind="ExternalInput")
    cwm_d = nc.dram_tensor("cwm", [128, 2560], BF16, kind="ExternalInput")
    cf_d = nc.dram_tensor("cf", [128, 258], F32, kind="ExternalInput")
    y_d = nc.dram_tensor("y", [tq, D], F32, kind="ExternalOutput")

    with tile.TileContext(nc) as tc:
        with (
            tc.tile_pool(name="persist", bufs=1) as pp,
            tc.tile_pool(name="vpool", bufs=1) as vp,
            tc.tile_pool(name="ptp", bufs=pt_bufs) as ptp,
            tc.tile_pool(name="outp", bufs=3) as outp,
            tc.tile_pool(name="finp", bufs=4) as finp,
        ):
            cwm = pp.tile([128, 2560], BF16, name="cwm")
            cf = pp.tile([128, 258], F32, name="cf")
            xTa = pp.tile([128, 2 * t], BF16, name="xTa")
            xqTa = pp.tile([128, 2 * tq], BF16, name="xqTa")
            xTv = xTa[:].rearrange("p (k c) -> p k c", k=2)
            xqTv = xqTa[:].rearrange("p (k c) -> p k c", k=2)
            xT = [xTa[:, k * t : (k + 1) * t] for k in range(2)]
            xqT = [xqTa[:, k * tq : (k + 1) * tq] for k in range(2)]

            def xts(k, a, b):
                return xT[k][:, a:b]

            def xqs(k, a, b):
                return xqT[k][:, a:b]
            KT8 = pp.tile([128, 2 * t], FP8, name="KT8")
            QT8 = pp.tile([128, 2 * tq], FP8, name="QT8")
            KT8v = KT8[:].rearrange("p (m c) -> p m c", m=2)
            QT8v = QT8[:].rearrange("p (m c) -> p m c", m=2)
            V = [vp.tile([128, D + 1], BF16, name=f"v{s}") for s in range(ns)]

            nc.scalar.dma_start(xqTv[:, :, 0:512], xqT_d[:, :, 0:512])
            nc.scalar.dma_start(cf[:], cf_d[:])
            nc.sync.dma_start(cwm[:], cwm_d[:])
            nc.sync.dma_start(xTv[:, :, 0:1024], xT_d[:, :, 0:1024])
            nc.sync.dma_start(xTv[:, :, 1024:2048], xT_d[:, :, 1024:2048])
            nc.sync.dma_start(xqTv[:, :, 512:1024], xqT_d[:, :, 512:1024])
            nc.sync.dma_start(xTv[:, :, 2048:3072], xT_d[:, :, 2048:3072])
            nc.sync.dma_start(xqTv[:, :, 1024:tq], xqT_d[:, :, 1024:tq])
            nc.sync.dma_start(xTv[:, :, 3072:t], xT_d[:, :, 3072:t])

            wq = [cwm[:, 0 + k * 256 : 256 + k * 256] for k in range(2)]
            wk = [cwm[:, 512 + k * 256 : 768 + k * 256] for k in range(2)]
            wv = [cwm[:, 1024 + k * 256 : 1280 + k * 256] for k in range(2)]
            maskb = [cwm[:, 1536 + r * 128 : 1536 + (r + 1) * 128] for r in range(8)]
            bq = [cf[:, k : k + 1] for k in range(2)]
            bvb = cf[:, 2:258]

            wa = pp.tile([128, 128], BF16, name="wa")
            wb = pp.tile([128, 512], BF16, name="wb")
            nc.vector.memset(wa[:], 0.0)
            nc.vector.memset(wb[:], 0.0)
            with tc.tile_pool(name="warm_ps", bufs=1, space="PSUM") as wps:
                wp_t = wps.tile([128, 512], F32, name="warm")
                for _ in range(warm_n):
                    nc.tensor.matmul(wp_t[:], wa[:], wb[:], start=True, stop=True)

            # ones columns on Pool (keeps DVE/ACT clear)
            for s in range(ns):
                nc.gpsimd.memset(V[s][:, D : D + 1], 1.0)
            # absorber copies pull const DMA completions into each consumer
            # engine's vector clock (single extra sem wait per consumer op)
            scrb = finp.tile([128, 1], BF16, name="scrb", tag="scrb")
            nc.vector.tensor_copy(scrb[:], cwm[:, 0:1])
            scrf = finp.tile([128, 1], F32, name="scrf", tag="scrf")
            nc.vector.tensor_copy(scrf[:], cf[:, 0:1])
            scrg = finp.tile([128, 1], F32, name="scrg", tag="scrg")
            nc.scalar.copy(scrg[:], cf[:, 0:1])
            scrp = finp.tile([128, 1], BF16, name="scrp", tag="scrp")
            nc.gpsimd.tensor_copy(scrp[:], cwm[:, 1024:1025])

            exp_t = mybir.ActivationFunctionType.Exp
            with (
                tc.tile_pool(name="st_ps", bufs=st_bufs, space="PSUM") as st_ps,
                tc.tile_pool(name="o_ps", bufs=o_bufs, space="PSUM") as o_ps_pool,
            ):
                def q_proj(g):
                    for m in range(2):
                        ms = slice(m * 128, (m + 1) * 128)
                        ps = o_ps_pool.tile([128, 512], F32, name="pj", tag="o")
                        for k in range(2):
                            nc.tensor.matmul(
                                ps[:], wq[k][:, ms],
                                xqs(k, g * 512, (g + 1) * 512),
                                start=(k == 0), stop=(k == 1),
                            )
                        dst = QT8[:, m * tq + g * 512 : m * tq + (g + 1) * 512]
                        nc.vector.tensor_scalar_add(dst, ps[:], bq[m])

                def k_proj(g):
                    # no bias: q·bk is row-constant, softmax-invariant
                    for nb in range(2):
                        ca = g * 1024 + nb * 512
                        for m in range(2):
                            ms = slice(m * 128, (m + 1) * 128)
                            ps = o_ps_pool.tile([128, 512], F32, name="pj", tag="o")
                            for k in range(2):
                                nc.tensor.matmul(
                                    ps[:], wk[k][:, ms], xts(k, ca, ca + 512),
                                    start=(k == 0), stop=(k == 1),
                                )
                            dst = KT8[
                                :,
                                m * t + g * 1024 + nb * 512 :
                                m * t + g * 1024 + (nb + 1) * 512,
                            ]
                            nc.vector.tensor_copy(dst, ps[:])

                def v_proj(g):
                    # bias-free; bv is added in the finalize
                    for s in range(8 * g, 8 * g + 8):
                        ps = o_ps_pool.tile([128, D], F32, name="vpj", tag="o")
                        for k in range(2):
                            nc.tensor.matmul(
                                ps[:], xts(k, s * 128, (s + 1) * 128), wv[k],
                                start=(k == 0), stop=(k == 1),
                            )
                        # GPSIMD cannot access PSUM -> DVE
                        nc.vector.tensor_copy(V[s][:, 0:D], ps[:])

                for g in range(ng):
                    e = 8 * g + 8
                    q_proj(g), k_proj(g), v_proj(g)
                    if g == 0:
                        scrm = finp.tile([128, 1], BF16, name="scrm", tag="scrm")
                        nc.vector.tensor_copy(scrm[:], cwm[:, 1536:1537])
                    # ---- attention for group g
                    o = [
                        o_ps_pool.tile([128, D + 1], F32, name=f"o{u}", tag="o")
                        for u in range(4)
                    ]
                    for s in range(e):
                        s_rel = s - 8 * g
                        u0 = max(s_rel, 0) // 2
                        c0 = u0 * 128
                        stp = st_ps.tile([128, 512], F32, name="stp", tag="stp")
                        nc.tensor.matmul(
                            stp[:, c0:512],
                            KT8v[:, :, s * 128 : (s + 1) * 128],
                            QT8v[:, :, g * 512 + c0 : (g + 1) * 512],
                            start=True, stop=True,
                            perf_mode=mybir.MatmulPerfMode.DoubleRow,
                        )
                        pt = ptp.tile([128, 512], BF16, name="pt", tag="pt")
                        nc.scalar.activation(
                            pt[:, c0:512], stp[:, c0:512], exp_t, scale=scale
                        )
                        if s_rel >= 0:
                            nc.vector.tensor_mul(
                                pt[:, c0 : c0 + 128],
                                pt[:, c0 : c0 + 128],
                                maskb[s_rel],
                            )
                        # u0's P block waits on the DVE mask -> run it last
                        for u in list(range(u0 + 1, 4)) + [u0]:
                            nc.tensor.matmul(
                                o[u][:],
                                pt[:, u * 128 : (u + 1) * 128],
                                V[s][:],
                                start=(s == 0),
                                stop=(s == 8 * g + 2 * u + 1),
                            )
                            if s == 8 * g + 2 * u + 1:
                                rec = finp.tile([128, 1], F32, name="rec", tag="rec")
                                nc.vector.reciprocal(rec[:], o[u][:, D : D + 1])
                                ob = outp.tile([128, D], F32, name="ob", tag="ob")
                                nc.vector.scalar_tensor_tensor(
                                    ob[:], o[u][:, 0:D], rec[:], bvb,
                                    mybir.AluOpType.mult, mybir.AluOpType.add,
                                )
                                lrow = (g * 4 + u) * 128
                                nc.sync.dma_start(y_d[lrow : lrow + 128, :], ob[:])
    return nc


def build_nc_v6(
    t: int = T,
    tq: int = TQ,
    st_bufs: int = 3,
    o_bufs: int = 5,
    pt_bufs: int = 6,
    warm_n: int = 12,
) -> bass.Bass:
    """v5 + QK-fold: the K projection is algebraically eliminated.

    S = (xq Wq + bq)(x Wk)^T  [bk dropped: softmax-invariant]
      = xq (Wq Wk^T) x^T + (x Wk bq)^T-broadcast
      = Yq x^T + beta_s
    with M = Wq Wk^T and c = Wk bq folded on the HOST (weight-only /
    thin matvec), beta_s lands in the exp's per-partition bias operand
    (out = exp(in*scale + bias), bias pre-scaled by `scale` host-side).
    The kernel computes Yq = xq M on PE (bf16, same cost as the old Q
    projection), casts to fp8, and runs S^T = DoubleRow(x8T, Yq8) with
    the host-quantized fp8 copy of x as the stationary side. Precision
    is unchanged vs v4/v5 (x8 plays k8's role, Yq8 plays q8's).

    PE col count drops from ~139k to ~123k (K proj gone), DVE loses the
    16 K-cast ops, ACT stays exp-only.
    """
    nq = tq // 128
    ns = t // 128
    ng = nq // 4
    assert t == 2 * tq and nq % 4 == 0
    scale = 1.0 / float(np.sqrt(np.float32(D)))
    FP8 = mybir.dt.float8e4

    nc = bacc.Bacc()
    xT_d = nc.dram_tensor("xT", [128, 2, t], BF16, kind="ExternalInput")
    x8T_d = nc.dram_tensor("x8T", [128, 2, t], FP8, kind="ExternalInput")
    xqT_d = nc.dram_tensor("xqT", [128, 2, tq], BF16, kind="ExternalInput")
    cwm_d = nc.dram_tensor("cwm", [128, 2048], BF16, kind="ExternalInput")
    cf_d = nc.dram_tensor("cf", [128, 288], F32, kind="ExternalInput")
    y_d = nc.dram_tensor("y", [tq, D], F32, kind="ExternalOutput")

    with tile.TileContext(nc) as tc:
        with (
            tc.tile_pool(name="persist", bufs=1) as pp,
            tc.tile_pool(name="vpool", bufs=1) as vp,
            tc.tile_pool(name="ptp", bufs=pt_bufs) as ptp,
            tc.tile_pool(name="outp", bufs=3) as outp,
            tc.tile_pool(name="finp", bufs=4) as finp,
        ):
            cwm = pp.tile([128, 2048], BF16, name="cwm")
            cf = pp.tile([128, 288], F32, name="cf")
            xTa = pp.tile([128, 2 * t], BF16, name="xTa")
            x8Ta = pp.tile([128, 2 * t], FP8, name="x8Ta")
            xqTa = pp.tile([128, 2 * tq], BF16, name="xqTa")
            xTv = xTa[:].rearrange("p (k c) -> p k c", k=2)
            x8Tv = x8Ta[:].rearrange("p (k c) -> p k c", k=2)
            xqTv = xqTa[:].rearrange("p (k c) -> p k c", k=2)
            xT = [xTa[:, k * t : (k + 1) * t] for k in range(2)]
            xqT = [xqTa[:, k * tq : (k + 1) * tq] for k in range(2)]

            def xts(k, a, b):
                return xT[k][:, a:b]

            def xqs(k, a, b):
                return xqT[k][:, a:b]
            Yq8 = pp.tile([128, 2 * tq], FP8, name="Yq8")
            Yq8v = Yq8[:].rearrange("p (m c) -> p m c", m=2)
            V = [vp.tile([128, D + 1], BF16, name=f"v{s}") for s in range(ns)]

            # ---- input DMA, need-ordered. ACT-side queue: group-0 xq +
            # constants; SP queue: everything else in group order.
            nc.scalar.dma_start(xqTv[:, :, 0:512], xqT_d[:, :, 0:512])
            nc.scalar.dma_start(cf[:], cf_d[:])
            nc.sync.dma_start(cwm[:], cwm_d[:])
            nc.sync.dma_start(x8Tv[:, :, 0:1024], x8T_d[:, :, 0:1024])
            nc.sync.dma_start(xTv[:, :, 0:1024], xT_d[:, :, 0:1024])
            nc.sync.dma_start(xTv[:, :, 1024:2048], xT_d[:, :, 1024:2048])
            nc.sync.dma_start(x8Tv[:, :, 1024:2048], x8T_d[:, :, 1024:2048])
            nc.sync.dma_start(xqTv[:, :, 512:1024], xqT_d[:, :, 512:1024])
            nc.sync.dma_start(xTv[:, :, 2048:3072], xT_d[:, :, 2048:3072])
            nc.sync.dma_start(x8Tv[:, :, 2048:3072], x8T_d[:, :, 2048:3072])
            nc.sync.dma_start(xqTv[:, :, 1024:tq], xqT_d[:, :, 1024:tq])
            nc.sync.dma_start(xTv[:, :, 3072:t], xT_d[:, :, 3072:t])
            nc.sync.dma_start(x8Tv[:, :, 3072:t], x8T_d[:, :, 3072:t])

            Mh = [cwm[:, 0 + k * 256 : 256 + k * 256] for k in range(2)]
            wv = [cwm[:, 512 + k * 256 : 768 + k * 256] for k in range(2)]
            maskb = [cwm[:, 1024 + r * 128 : 1024 + (r + 1) * 128] for r in range(8)]
            beta = [cf[:, s : s + 1] for s in range(ns)]
            bvb = cf[:, 32:288]

            # warm-up garbage matmuls bridge the input-DMA wait so the PE
            # p-state/clock gate is hot when real work arrives
            wa = pp.tile([128, 128], BF16, name="wa")
            wb = pp.tile([128, 512], BF16, name="wb")
            nc.vector.memset(wa[:], 0.0)
            nc.vector.memset(wb[:], 0.0)
            with tc.tile_pool(name="warm_ps", bufs=1, space="PSUM") as wps:
                wp_t = wps.tile([128, 512], F32, name="warm")
                for _ in range(warm_n):
                    nc.tensor.matmul(wp_t[:], wa[:], wb[:], start=True, stop=True)

            # ones columns (denominator trick) on Pool, off everyone's path
            for s in range(ns):
                nc.gpsimd.memset(V[s][:, D : D + 1], 1.0)
            # absorber copies pull const DMA completions into each consumer
            # engine's vector clock (single extra sem wait per consumer op)
            scrb = finp.tile([128, 1], BF16, name="scrb", tag="scrb")
            nc.vector.tensor_copy(scrb[:], cwm[:, 0:1])
            scrf = finp.tile([128, 1], F32, name="scrf", tag="scrf")
            nc.vector.tensor_copy(scrf[:], cf[:, 0:1])
            scrg = finp.tile([128, 1], F32, name="scrg", tag="scrg")
            nc.scalar.copy(scrg[:], cf[:, 0:1])

            exp_t = mybir.ActivationFunctionType.Exp
            with (
                tc.tile_pool(name="st_ps", bufs=st_bufs, space="PSUM") as st_ps,
                tc.tile_pool(name="o_ps", bufs=o_bufs, space="PSUM") as o_ps_pool,
            ):
                def yq_proj(g):
                    # Yq = xq M, no bias (bq lives in beta); fp8 cast on DVE
                    for m in range(2):
                        ms = slice(m * 128, (m + 1) * 128)
                        ps = o_ps_pool.tile([128, 512], F32, name="pj", tag="o")
                        for k in range(2):
                            nc.tensor.matmul(
                                ps[:], Mh[k][:, ms],
                                xqs(k, g * 512, (g + 1) * 512),
                                start=(k == 0), stop=(k == 1),
                            )
                        dst = Yq8[:, m * tq + g * 512 : m * tq + (g + 1) * 512]
                        nc.vector.tensor_copy(dst, ps[:])

                def v_proj(g):
                    # bias-free; bv is added in the finalize
                    for s in range(8 * g, 8 * g + 8):
                        ps = o_ps_pool.tile([128, D], F32, name="vpj", tag="o")
                        for k in range(2):
                            nc.tensor.matmul(
                                ps[:], xts(k, s * 128, (s + 1) * 128), wv[k],
                                start=(k == 0), stop=(k == 1),
                            )
                        nc.vector.tensor_copy(V[s][:, 0:D], ps[:])

                for g in range(ng):
                    e = 8 * g + 8
                    yq_proj(g), v_proj(g)
                    if g == 0:
                        scrm = finp.tile([128, 1], BF16, name="scrm", tag="scrm")
                        nc.vector.tensor_copy(scrm[:], cwm[:, 1024:1025])
                    # ---- attention for group g
                    o = [
                        o_ps_pool.tile([128, D + 1], F32, name=f"o{u}", tag="o")
                        for u in range(4)
                    ]
                    for s in range(e):
                        s_rel = s - 8 * g
                        u0 = max(s_rel, 0) // 2
                        c0 = u0 * 128
                        stp = st_ps.tile([128, 512], F32, name="stp", tag="stp")
                        nc.tensor.matmul(
                            stp[:, c0:512],
                            x8Tv[:, :, s * 128 : (s + 1) * 128],
                            Yq8v[:, :, g * 512 + c0 : (g + 1) * 512],
                            start=True, stop=True,
                            perf_mode=mybir.MatmulPerfMode.DoubleRow,
                        )
                        pt = ptp.tile([128, 512], BF16, name="pt", tag="pt")
                        nc.scalar.activation(
                            pt[:, c0:512], stp[:, c0:512], exp_t,
                            bias=beta[s], scale=scale,
                        )
                        if s_rel >= 0:
                            nc.vector.tensor_mul(
                                pt[:, c0 : c0 + 128],
                                pt[:, c0 : c0 + 128],
                                maskb[s_rel],
                            )
                        # u0's P block waits on the DVE mask -> run it last
                        for u in list(range(u0 + 1, 4)) + [u0]:
                            nc.tensor.matmul(
                                o[u][:],
                                pt[:, u * 128 : (u + 1) * 128],
                                V[s][:],
                                start=(s == 0),
                                stop=(s == 8 * g + 2 * u + 1),
                            )
                            if s == 8 * g + 2 * u + 1:
                                rec = finp.tile([128, 1], F32, name="rec", tag="rec")
                                nc.vector.reciprocal(rec[:], o[u][:, D : D + 1])
                                ob = outp.tile([128, D], F32, name="ob", tag="ob")
                                nc.vector.scalar_tensor_tensor(
                                    ob[:], o[u][:, 0:D], rec[:], bvb,
                                    mybir.AluOpType.mult, mybir.AluOpType.add,
                                )
                                lrow = (g * 4 + u) * 128
                                nc.sync.dma_start(y_d[lrow : lrow + 128, :], ob[:])
    return nc


def build_nc_v7(
    t: int = T,
    tq: int = TQ,
    st_bufs: int = 3,
    o_bufs: int = 5,
    pt_bufs: int = 6,
    warm_n: int = 10,
) -> bass.Bass:
    """v6 + pipeline surgery driven by the v6 trace:

    - Input DMA all on the fast SP queue in true need order; cwm is split
      so M+Wv (needed first) are not stuck behind the 256KB of masks.
      xq group 0 leads (v6 had it on the ACT queue, which started ~3us
      late and delayed the first Yq projection to t=12.6us).
    - V projections+casts are INTERLEAVED into the s-loop (tile 8g+2+j
      lands at s-loop position j) instead of a block before each group:
      the casts no longer monopolize DVE's in-order queue right when the
      masks need it (v6 lost 3.8us at the g=0 boundary to this), and the
      proj matmuls fill PE's exp-wait bubbles.
    - Yq projection for group g+1 is emitted near the tail of group g's
      s-loop, so its fp8 cast completes before g+1's first S^T.
    - Causal 0/1 masks run on GpSimd (SBUF-only op, engine is idle)
      removing them from DVE's queue entirely.
    """
    nq = tq // 128
    ns = t // 128
    ng = nq // 4
    assert t == 2 * tq and nq % 4 == 0
    scale = 1.0 / float(np.sqrt(np.float32(D)))
    FP8 = mybir.dt.float8e4

    nc = bacc.Bacc()
    xT_d = nc.dram_tensor("xT", [128, 2, t], BF16, kind="ExternalInput")
    x8T_d = nc.dram_tensor("x8T", [128, 2, t], FP8, kind="ExternalInput")
    xqT_d = nc.dram_tensor("xqT", [128, 2, tq], BF16, kind="ExternalInput")
    cw_d = nc.dram_tensor("cw", [128, 1024], BF16, kind="ExternalInput")
    cm_d = nc.dram_tensor("cm", [128, 512], BF16, kind="ExternalInput")
    cf_d = nc.dram_tensor("cf", [128, 32], F32, kind="ExternalInput")
    y_d = nc.dram_tensor("y", [tq, D], F32, kind="ExternalOutput")

    with tile.TileContext(nc) as tc:
        with (
            tc.tile_pool(name="persist", bufs=1) as pp,
            tc.tile_pool(name="vpool", bufs=1) as vp,
            tc.tile_pool(name="ptp", bufs=pt_bufs) as ptp,
            tc.tile_pool(name="outp", bufs=3) as outp,
            tc.tile_pool(name="finp", bufs=4) as finp,
        ):
            cw = pp.tile([128, 1024], BF16, name="cw")
            cm = pp.tile([128, 512], BF16, name="cm")
            cf = pp.tile([128, 32], F32, name="cf")
            xTa = pp.tile([128, 2 * t], BF16, name="xTa")
            x8Ta = pp.tile([128, 2 * t], FP8, name="x8Ta")
            xqTa = pp.tile([128, 2 * tq], BF16, name="xqTa")
            xTv = xTa[:].rearrange("p (k c) -> p k c", k=2)
            x8Tv = x8Ta[:].rearrange("p (k c) -> p k c", k=2)
            xqTv = xqTa[:].rearrange("p (k c) -> p k c", k=2)
            xT = [xTa[:, k * t : (k + 1) * t] for k in range(2)]
            xqT = [xqTa[:, k * tq : (k + 1) * tq] for k in range(2)]

            def xts(k, a, b):
                return xT[k][:, a:b]

            def xqs(k, a, b):
                return xqT[k][:, a:b]
            Yq8 = pp.tile([128, 2 * tq], FP8, name="Yq8")
            Yq8v = Yq8[:].rearrange("p (m c) -> p m c", m=2)
            V = [vp.tile([128, D + 1], BF16, name=f"v{s}") for s in range(ns)]

            # ---- input DMA: single fast SP queue, true need order. The
            # group-0 critical set (M, xq g0, x8T g0, beta, first xT rows,
            # masks) leads; everything else streams behind it in the order
            # the interleaved projections consume it.
            nc.sync.dma_start(cw[:], cw_d[:])
            nc.sync.dma_start(xqTv[:, :, 0:512], xqT_d[:, :, 0:512])
            nc.sync.dma_start(cf[:], cf_d[:])
            nc.sync.dma_start(x8Tv[:, :, 0:1024], x8T_d[:, :, 0:1024])
            nc.sync.dma_start(xTv[:, :, 0:512], xT_d[:, :, 0:512])
            nc.sync.dma_start(cm[:], cm_d[:])
            nc.sync.dma_start(xTv[:, :, 512:1024], xT_d[:, :, 512:1024])
            nc.sync.dma_start(xqTv[:, :, 512:1024], xqT_d[:, :, 512:1024])
            nc.sync.dma_start(xTv[:, :, 1024:2048], xT_d[:, :, 1024:2048])
            nc.sync.dma_start(x8Tv[:, :, 1024:2048], x8T_d[:, :, 1024:2048])
            nc.sync.dma_start(xqTv[:, :, 1024:tq], xqT_d[:, :, 1024:tq])
            nc.sync.dma_start(xTv[:, :, 2048:3072], xT_d[:, :, 2048:3072])
            nc.sync.dma_start(x8Tv[:, :, 2048:3072], x8T_d[:, :, 2048:3072])
            nc.sync.dma_start(xTv[:, :, 3072:t], xT_d[:, :, 3072:t])
            nc.sync.dma_start(x8Tv[:, :, 3072:t], x8T_d[:, :, 3072:t])

            Mh = [cw[:, 0 + k * 256 : 256 + k * 256] for k in range(2)]
            wv = [cw[:, 512 + k * 256 : 768 + k * 256] for k in range(2)]
            # per-core mask data: block 0 = even s_rel, block 1 = odd s_rel
            maskb = [cm[:, (r % 2) * 128 : (r % 2) * 128 + 128] for r in range(8)]
            bvb = cm[:, 256:512]  # bf16 broadcast of bv (finalize add)
            beta = [cf[:, s : s + 1] for s in range(ns)]

            # warm-up garbage matmuls bridge the input-DMA wait so the PE
            # p-state/clock gate is hot when real work arrives
            wa = pp.tile([128, 128], BF16, name="wa")
            wb = pp.tile([128, 512], BF16, name="wb")
            nc.vector.memset(wa[:], 0.0)
            nc.vector.memset(wb[:], 0.0)
            with tc.tile_pool(name="warm_ps", bufs=1, space="PSUM") as wps:
                wp_t = wps.tile([128, 512], F32, name="warm")
                for _ in range(warm_n):
                    nc.tensor.matmul(wp_t[:], wa[:], wb[:], start=True, stop=True)

            # ones columns (denominator trick) on Pool, off everyone's path
            for s in range(ns):
                nc.gpsimd.memset(V[s][:, D : D + 1], 1.0)
            # absorber copies pull const DMA completions into each consumer
            # engine's vector clock (single extra sem wait per consumer op):
            # DVE reads cm (bvb in the finalize STT), ACT reads cf (exp
            # bias), GpSimd reads cm (masks)
            scrf = finp.tile([128, 1], BF16, name="scrf", tag="scrf")
            nc.vector.tensor_copy(scrf[:], cm[:, 256:257])
            scrg = finp.tile([128, 1], F32, name="scrg", tag="scrg")
            nc.scalar.copy(scrg[:], cf[:, 0:1])
            scrm = finp.tile([128, 1], BF16, name="scrm", tag="scrm")
            nc.gpsimd.tensor_copy(scrm[:], cm[:, 0:1])

            exp_t = mybir.ActivationFunctionType.Exp
            with (
                tc.tile_pool(name="st_ps", bufs=st_bufs, space="PSUM") as st_ps,
                tc.tile_pool(name="o_ps", bufs=o_bufs, space="PSUM") as o_ps_pool,
            ):
                def yq_proj(g, m):
                    # Yq = xq M, no bias (bq lives in beta); fp8 cast on DVE
                    ms = slice(m * 128, (m + 1) * 128)
                    ps = o_ps_pool.tile([128, 512], F32, name="pj", tag="o")
                    for k in range(2):
                        nc.tensor.matmul(
                            ps[:], Mh[k][:, ms],
                            xqs(k, g * 512, (g + 1) * 512),
                            start=(k == 0), stop=(k == 1),
                        )
                    dst = Yq8[:, m * tq + g * 512 : m * tq + (g + 1) * 512]
                    nc.vector.tensor_copy(dst, ps[:])

                def v_proj(s):
                    # bias-free; bv is added in the finalize
                    ps = o_ps_pool.tile([128, D], F32, name="vpj", tag="o")
                    for k in range(2):
                        nc.tensor.matmul(
                            ps[:], xts(k, s * 128, (s + 1) * 128), wv[k],
                            start=(k == 0), stop=(k == 1),
                        )
                    nc.vector.tensor_copy(V[s][:, 0:D], ps[:])

                # group-0 lead-in: Yq(0) + first two V tiles
                yq_proj(0, 0), yq_proj(0, 1)
                v_proj(0), v_proj(1)

                for g in range(ng):
                    e = 8 * g + 8
                    o = [
                        o_ps_pool.tile([128, D + 1], F32, name=f"o{u}", tag="o")
                        for u in range(4)
                    ]
                    for s in range(e):
                        s_rel = s - 8 * g
                        u0 = max(s_rel, 0) // 2
                        c0 = u0 * 128
                        stp = st_ps.tile([128, 512], F32, name="stp", tag="stp")
                        nc.tensor.matmul(
                            stp[:, c0:512],
                            x8Tv[:, :, s * 128 : (s + 1) * 128],
                            Yq8v[:, :, g * 512 + c0 : (g + 1) * 512],
                            start=True, stop=True,
                            perf_mode=mybir.MatmulPerfMode.DoubleRow,
                        )
                        pt = ptp.tile([128, 512], BF16, name="pt", tag="pt")
                        nc.scalar.activation(
                            pt[:, c0:512], stp[:, c0:512], exp_t,
                            bias=beta[s], scale=scale,
                        )
                        if s_rel >= 0:
                            nc.gpsimd.tensor_mul(
                                pt[:, c0 : c0 + 128],
                                pt[:, c0 : c0 + 128],
                                maskb[s_rel],
                            )
                        # u0's P block waits on the mask -> run it last
                        for u in list(range(u0 + 1, 4)) + [u0]:
                            nc.tensor.matmul(
                                o[u][:],
                                pt[:, u * 128 : (u + 1) * 128],
                                V[s][:],
                                start=(s == 0),
                                stop=(s == 8 * g + 2 * u + 1),
                            )
                            if s == 8 * g + 2 * u + 1:
                                rec = finp.tile([128, 1], F32, name="rec", tag="rec")
                                nc.vector.reciprocal(rec[:], o[u][:, D : D + 1])
                                ob = outp.tile([128, D], F32, name="ob", tag="ob")
                                nc.vector.scalar_tensor_tensor(
                                    ob[:], o[u][:, 0:D], rec[:], bvb,
                                    mybir.AluOpType.mult, mybir.AluOpType.add,
                                )
                                lrow = (g * 4 + u) * 128
                                nc.sync.dma_start(y_d[lrow : lrow + 128, :], ob[:])
                        # interleaved projections for upcoming work: the six
                        # remaining V tiles of this group's diagonal, then
                        # (near the tail) the next group's Yq and lead V pair
                        if s < 6:
                            v_proj(8 * g + 2 + s)
                        if g + 1 < ng:
                            if s == e - 4:
                                yq_proj(g + 1, 0)
                            elif s == e - 3:
                                yq_proj(g + 1, 1)
                            elif s == e - 2:
                                v_proj(8 * (g + 1))
                            elif s == e - 1:
                                v_proj(8 * (g + 1) + 1)
    return nc


def prep_inputs(
    x, Wq, bq, Wk, bk, Wv, bv, t: int = T, n_cores: int = N_CORES, version: int = 1
):
    """Per-core input maps (host-side shard / transpose / cast)."""
    x = np.asarray(x, dtype=np.float32)
    b_dim = x.shape[0]
    tq = t // 2
    nq = tq // 128
    shared = {}
    for name, w in (("wq", Wq), ("wk", Wk), ("wv", Wv)):
        shared[name] = np.ascontiguousarray(
            np.asarray(w, np.float32).astype(NPBF16).reshape(2, 128, D)
        )
    shared["bq"] = np.ascontiguousarray(
        np.asarray(bq, np.float32).reshape(2, 128, 1)
    )
    shared["bk"] = np.ascontiguousarray(
        np.asarray(bk, np.float32).reshape(2, 128, 1)
    )
    shared["bvb"] = np.ascontiguousarray(
        np.broadcast_to(np.asarray(bv, np.float32), (128, D))
    )
    if version >= 6:
        return _prep_inputs_v6(x, Wq, bq, Wk, bk, Wv, bv, t, n_cores, version)
    idx = np.arange(128)
    tri = np.where(idx[:, None] > idx[None, :], np.float32(NEG), np.float32(0.0))
    full = np.full((128, 128), NEG, np.float32)
    zero = np.zeros((128, 128), np.float32)
    if version == 1:
        masks = [
            np.ascontiguousarray(np.concatenate([tri, full], axis=1)),
            np.ascontiguousarray(np.concatenate([zero, tri], axis=1)),
        ]
    elif version == 2:
        masks = []
        for h in range(2):
            m = np.empty((8, 128, 512), np.float32)
            for s_rel in range(8):
                for u in range(4):
                    blk = full if s_rel > 2 * u + h else (tri if s_rel == 2 * u + h else zero)
                    m[s_rel, :, u * 128 : (u + 1) * 128] = blk
            masks.append(np.ascontiguousarray(m))
    else:
        # v3/v4: multiplicative 0/1 bf16 masks, one 128-block per diag s_rel.
        # s_rel even -> block u0=s_rel/2: h=0 diag (keep s<=q), h=1 keep-all
        # s_rel odd  -> block u0:         h=0 dead (zeros),     h=1 diag
        tri01 = (idx[:, None] <= idx[None, :]).astype(NPBF16)
        ones = np.ones((128, 128), NPBF16)
        zeros = np.zeros((128, 128), NPBF16)
        masks = []
        for h in range(2):
            m = np.empty((8, 128, 128), NPBF16)
            for s_rel in range(8):
                if s_rel % 2 == 0:
                    m[s_rel] = tri01 if h == 0 else ones
                else:
                    m[s_rel] = zeros if h == 0 else tri01
            masks.append(m)
        if version >= 4:
            # v4: bk dropped (softmax-invariant) -> cf [128, 258] f32
            cf = np.empty((128, 258), np.float32)
            cf[:, 0:2] = np.asarray(bq, np.float32).reshape(2, 128).T
            cf[:, 2:258] = np.broadcast_to(np.asarray(bv, np.float32), (128, D))
        else:
            # pack constants: cw [128, 1536] bf16, cf [128, 260] f32
            cf = np.empty((128, 260), np.float32)
            cf[:, 0:2] = np.asarray(bq, np.float32).reshape(2, 128).T
            cf[:, 2:4] = np.asarray(bk, np.float32).reshape(2, 128).T
            cf[:, 4:260] = np.broadcast_to(np.asarray(bv, np.float32), (128, D))
        cw = np.empty((128, 1536), NPBF16)
        for j, w in enumerate((Wq, Wk, Wv)):
            wb = np.asarray(w, np.float32).astype(NPBF16).reshape(2, 128, D)
            cw[:, j * 512 : j * 512 + 256] = wb[0]
            cw[:, j * 512 + 256 : j * 512 + 512] = wb[1]
        cms = []
        for h in range(2):
            cm = np.empty((128, 1024), NPBF16)
            for r in range(8):
                cm[:, r * 128 : (r + 1) * 128] = masks[h][r]
            cms.append(np.ascontiguousarray(cm))
        if version >= 4:
            # v4: weights+masks in one bf16 tensor; x/xq as [128, k, cols]
            cwms = [
                np.ascontiguousarray(np.concatenate([cw, cms[h]], axis=1))
                for h in range(2)
            ]
        shared = {"cf": np.ascontiguousarray(cf), "cw": np.ascontiguousarray(cw)}
    in_maps = []
    for c in range(n_cores):
        b, h = divmod(c, 2)
        xb = x[b % b_dim]  # [t, D]
        xT = np.ascontiguousarray(xb.T.astype(NPBF16).reshape(2, 128, t))
        qrows = np.concatenate(
            [xb[g * 128 : (g + 1) * 128] for g in _qtiles(nq, h, version)], axis=0
        )
        xqT = np.ascontiguousarray(qrows.T.astype(NPBF16).reshape(2, 128, tq))
        if version >= 4:
            in_maps.append({
                "xT": np.ascontiguousarray(xT.transpose(1, 0, 2)),
                "xqT": np.ascontiguousarray(xqT.transpose(1, 0, 2)),
                "cwm": cwms[h],
                "cf": shared["cf"],
            })
        elif version == 3:
            in_maps.append({"xT": xT, "xqT": xqT, "cm": cms[h], **shared})
        else:
            in_maps.append({"xT": xT, "xqT": xqT, "mask": masks[h], **shared})
    return in_maps


NPFP8 = ml_dtypes.float8_e4m3fn


def _prep_inputs_v6(x, Wq, bq, Wk, bk, Wv, bv, t: int, n_cores: int,
                    version: int = 6):
    """Host prep for v6/v7 (QK-fold). Weight-only folds on host:
    M = Wq Wk^T, c = Wk bq; per-batch thin matvec beta = x c (f32),
    pre-scaled by 1/sqrt(D) for the exp bias operand. x is shipped in
    bf16 (V proj) AND as an fp8 copy (stationary side of S^T).

    v7 packs constants tighter: cw [128,1024] bf16 = [M halves | Wv
    halves]; cm [128,512] bf16 = [mask even | mask odd | bvb]; cf
    [128,32] f32 = beta only."""
    x = np.asarray(x, dtype=np.float32)
    b_dim = x.shape[0]
    tq = t // 2
    nq = tq // 128
    scale = np.float32(1.0 / np.sqrt(np.float32(D)))

    M = (np.asarray(Wq, np.float32) @ np.asarray(Wk, np.float32).T).astype(NPBF16)
    c = np.asarray(Wk, np.float32) @ np.asarray(bq, np.float32)
    wvb = np.asarray(Wv, np.float32).astype(NPBF16)
    bvb16 = np.broadcast_to(
        np.asarray(bv, np.float32).astype(NPBF16), (128, D)
    )

    # masks: multiplicative 0/1 bf16, one 128-block per diagonal s_rel.
    # Per core only two DISTINCT blocks exist: even s_rel and odd s_rel.
    idx = np.arange(128)
    tri01 = (idx[:, None] <= idx[None, :]).astype(NPBF16)
    ones = np.ones((128, 128), NPBF16)
    zeros = np.zeros((128, 128), NPBF16)
    mask_eo = [(tri01, zeros), (ones, tri01)]  # [h] -> (even, odd)

    cws, cms = [], []
    for h in range(2):
        if version >= 7:
            cw = np.zeros((128, 1024), NPBF16)
            cw[:, 0:256] = M[0:128]
            cw[:, 256:512] = M[128:256]
            cw[:, 512:768] = wvb[0:128]
            cw[:, 768:1024] = wvb[128:256]
            cm = np.zeros((128, 512), NPBF16)
            cm[:, 0:128] = mask_eo[h][0]
            cm[:, 128:256] = mask_eo[h][1]
            cm[:, 256:512] = bvb16
            cws.append(np.ascontiguousarray(cw))
            cms.append(np.ascontiguousarray(cm))
        else:
            cwm = np.zeros((128, 2048), NPBF16)
            cwm[:, 0:256] = M[0:128]
            cwm[:, 256:512] = M[128:256]
            cwm[:, 512:768] = wvb[0:128]
            cwm[:, 768:1024] = wvb[128:256]
            for s_rel in range(8):
                blk = mask_eo[h][s_rel % 2]
                cwm[:, 1024 + s_rel * 128 : 1024 + (s_rel + 1) * 128] = blk
            cws.append(np.ascontiguousarray(cwm))

    in_maps = []
    for cid in range(n_cores):
        b, h = divmod(cid, 2)
        xb = x[b % b_dim]  # [t, D] f32
        beta = (xb @ c).astype(np.float32) * scale  # [t]
        if version >= 7:
            cf = np.ascontiguousarray(beta.reshape(32, 128).T)
        else:
            cf = np.zeros((128, 288), np.float32)
            cf[:, 0:32] = beta.reshape(32, 128).T
            cf[:, 32:288] = np.broadcast_to(np.asarray(bv, np.float32), (128, D))
            cf = np.ascontiguousarray(cf)
        xTb = xb.T.astype(NPBF16).reshape(2, 128, t)  # [k, 128, t]
        xT = np.ascontiguousarray(xTb.transpose(1, 0, 2))
        x8T = np.ascontiguousarray(xTb.astype(NPFP8).transpose(1, 0, 2))
        qrows = np.concatenate(
            [xb[g * 128 : (g + 1) * 128] for g in _qtiles(nq, h, 6)], axis=0
        )
        xqT = np.ascontiguousarray(
            qrows.T.astype(NPBF16).reshape(2, 128, tq).transpose(1, 0, 2)
        )
        im = {"xT": xT, "x8T": x8T, "xqT": xqT, "cf": cf}
        if version >= 7:
            im["cw"] = cws[h]
            im["cm"] = cms[h]
        else:
            im["cwm"] = cws[h]
        in_maps.append(im)
    return in_maps


def _qtiles(nq: int, h: int, version: int) -> list[int]:
    """Global q-tile index for each local tile, in local order."""
    if version == 1:
        return [2 * i + h for i in range(nq)]
    return [8 * g + 2 * u + h for g in range(nq // 4) for u in range(4)]


_BUILDERS = {1: build_nc, 2: build_nc_v2, 3: build_nc_v3, 4: build_nc_v4, 5: build_nc_v5, 6: build_nc_v6, 7: build_nc_v7}


def gather_output(results, t: int = T, n_cores: int = N_CORES, version: int = 1):
    tq = t // 2
    nq = tq // 128
    y = np.empty((n_cores // 2, t, D), np.float32)
    for c in range(n_cores):
        b, h = divmod(c, 2)
        yc = np.asarray(results[c]["y"])
        for li, g in enumerate(_qtiles(nq, h, version)):
            y[b, g * 128 : (g + 1) * 128] = yc[li * 128 : (li + 1) * 128]
    return y


VERSION = 7


def run_on_hw(inputs: dict, trace: bool = False):
    """Returns (y [B,T,D] f32, BassKernelResults)."""
    in_maps = prep_inputs(**inputs, version=VERSION)
    nc = _BUILDERS[VERSION]()
    if not nc.is_finalized():
        nc.finalize()
    res = run_bass_kernel_spmd(nc, in_maps, list(range(N_CORES)), trace=trace)
    return gather_output(res.results, version=VERSION), res


def kernel(**inputs) -> np.ndarray:
    y, _ = run_on_hw(inputs, trace=False)
    return y

